# revision 1
# baseline (speedup 1.0000x reference)
"""Trainium2 Bass kernel: exact 3D Euclidean distance transform of a binary
(16, 512, 512) float32 volume — distance from every nonzero voxel to the
nearest zero voxel over ALL three axes (batch participates in the metric),
matching scipy.ndimage.distance_transform_edt on the full array.

Algorithm (separable min-plus with squared-parabola kernels; order-free):
  pass W: exact 1D nearest-zero distance along W via fwd/bwd saturating
          scans (tensor_tensor_scan, values clamp at CLAMP=32 == +inf),
          then square (PE transpose + ACT Square).
  pass B: banded parabola min-plus along B (|delta| <= R).
  pass H: banded parabola min-plus along H (|delta| <= R).
Banding is exact whenever the true max distance <= R; this is verified on
the host after the run (max over the output), with an exact host fallback
otherwise.  All intermediates are small integers (<= CLAMP^2 + R^2), exact
in fp16, which unlocks the DVE 2x perf mode.

Sharding: kernel 1 is data-parallel over H (8 slabs of 64 rows; the W-scan
and the B-pass need full W and full B, which each slab has).  The squared
intermediate is stored w-major; the host reshards it W-wise (numpy slicing)
and kernel 2 (data-parallel over W, 8 slabs of 64 cols) runs the H-pass,
which needs full H, plus the final sqrt.

Hardware quirk driving the structure: several instruction encodings
(DMA DIRECT2D, the S2S2D2 tensor-scalar-ptr family used by the scans and
scalar_tensor_tensor) accept only ONE semaphore wait.  Tile emits a wait
only when an engine's vector clock is behind, so the kernel is arranged so
every such instruction has at most one not-yet-observed cross-engine
dependency, and each kernel issues at most 8 HWDGE DMAs (no lane reuse).
"""
import numpy as np

B, H, W = 16, 512, 512
NCORES = 8
HS = H // NCORES
WS = W // NCORES
P = 128
CLAMP = 32.0
R = 5

_BUILT = None
LAST_RESULTS = []   # BassKernelResults of the most recent kernel() call


def _k1_body(tc, d2t_d, xs_d):
    """Pass W scans + PE transpose/ACT square + banded pass B.

    xs_d:  [16, HS, 512] f32 dram (ExternalInput)
    d2t_d: [512, 16, HS] f16 dram (ExternalOutput), squared distances

    Engine balance: both scans run on DVE (the bwd scan runs on the fwd
    result, which already equals min(fwd, d0), so its output is
    min(fwd, bwd) directly — no separate combine).  The banded pass-B adds
    are split between ACT (Copy+bias) and DVE (tensor_scalar); the mins run
    on DVE (tensor_tensor, 2x mode: every operand offset is a multiple of
    HS=64 fp16 elements, so alignment holds).
    """
    import concourse.mybir as mybir

    nc = tc.nc
    f16 = mybir.dt.float16
    f32 = mybir.dt.float32
    Alu = mybir.AluOpType
    Act = mybir.ActivationFunctionType
    N_T = (B * HS) // P       # 8 scan tiles
    N_J = W // P              # 4 w-groups
    C = B * HS                # 1024 free elements per w after transpose

    from concourse.masks import make_identity

    with tc.tile_pool(name="const", bufs=1) as cpool, \
         tc.tile_pool(name="big", bufs=1) as bpool, \
         tc.tile_pool(name="tmp", bufs=3) as tpool, \
         tc.tile_pool(name="psum", bufs=7, space="PSUM") as ppool, \
         tc.tile_pool(name="psumw", bufs=1, space="PSUM") as ppoolw:

        ident = cpool.tile([P, P], f16)
        make_identity(nc, ident[:])
        ones = cpool.tile([P, W], f16)
        nc.vector.memset(ones[:], 1.0)
        # dummy transpose so PE observes the gpsimd-built identity before the
        # real transposes (keeps every matmul at <= 1 semaphore wait)
        psw = ppoolw.tile([P, P], f16)
        nc.tensor.transpose(psw[:], ident[:], ident[:])

        XH = bpool.tile([P, N_T * W], f16)      # x cast to f16 by the DMA
        AALL = bpool.tile([P, N_T * W], f16)    # d0 = (x != 0) * CLAMP
        FALL = bpool.tile([P, N_T * W], f16)    # fwd scan
        DALL = bpool.tile([P, N_T * W], f16)    # bwd scan of fwd = 1D dist

        qs = N_T * W // 4
        for m in range(4):
            nc.gpsimd.dma_start(
                XH[:, qs * m: qs * (m + 1)].rearrange(
                    "p (g w) -> p g w", g=2),
                xs_d[4 * m: 4 * (m + 1)].rearrange(
                    "(g bb) h w -> (bb h) g w", g=2, bb=2))
        for q in range(4):
            qs = N_T * W // 4
            nc.vector.tensor_scalar(
                AALL[:, qs * q: qs * (q + 1)],
                XH[:, qs * q: qs * (q + 1)],
                0.0, CLAMP, Alu.not_equal, Alu.mult)

        for t in range(N_T):
            fa = FALL[:, W * t: W * (t + 1)]
            nc.vector.tensor_tensor_scan(
                fa, ones[:], AALL[:, W * t: W * (t + 1)], CLAMP,
                Alu.add, Alu.min)
            nc.vector.tensor_tensor_scan(
                DALL[:, W * t: W * (t + 1)][:, ::-1], ones[:], fa[:, ::-1],
                CLAMP, Alu.add, Alu.min)

        SQ = bpool.tile([P, N_J * C], f16)      # w lines x (j, b, h)
        for j in range(N_J):
            ps = ppool.tile([P, C], f16, tag="ps")
            for t in range(N_T):
                nc.tensor.transpose(
                    ps[:, P * t:P * (t + 1)],
                    DALL[:, W * t + P * j: W * t + P * (j + 1)], ident[:])
            nc.scalar.activation(SQ[:, C * j:C * (j + 1)], ps[:], Act.Square)

        # banded pass B: per shift, an add (ACT or DVE tensor_scalar) then a
        # DVE tensor_tensor min.  ACC is initialized by the first (s=1) min
        # for b<15 plus a small ACT copy for the b=15 strip.
        ACC = bpool.tile([P, N_J * C], f16)
        sq4 = SQ[:].rearrange("p (j b h) -> p j b h", j=N_J, b=B)
        ac4 = ACC[:].rearrange("p (j b h) -> p j b h", j=N_J, b=B)
        nc.scalar.activation(ac4[:, :, B - 1:B, :], sq4[:, :, B - 1:B, :],
                             Act.Copy, bias=0.0)
        first = True
        for s in range(1, R + 1):
            bc = B - s
            for sgn in (1, -1):
                if sgn > 0:
                    srcv = sq4[:, :, s:s + bc, :]
                    outv = ac4[:, :, 0:bc, :]
                else:
                    srcv = sq4[:, :, 0:bc, :]
                    outv = ac4[:, :, s:B, :]
                tmp = tpool.tile([P, N_J * C], f16, tag="tmp")
                tmpv = tmp[:].rearrange(
                    "p (j b h) -> p j b h", j=N_J, b=B)[:, :, 0:bc, :]
                if s >= 2:
                    nc.scalar.activation(tmpv, srcv, Act.Copy,
                                         bias=float(s * s))
                else:
                    nc.vector.tensor_scalar(tmpv, srcv, float(s * s), None,
                                            Alu.add)
                if first:
                    # acc := min(sq (delta=0), tmp) initializes b < 15
                    nc.vector.tensor_tensor(outv, tmpv, sq4[:, :, 0:bc, :],
                                            Alu.min)
                    first = False
                elif s == R and sgn == -1:
                    # split the final fold by j-halves so the two output DMAs
                    # can start as soon as their half is done
                    jh = N_J // 2
                    nc.vector.tensor_tensor(outv[:, 0:jh], tmpv[:, 0:jh],
                                            outv[:, 0:jh], Alu.min)
                    nc.vector.tensor_tensor(outv[:, jh:N_J], tmpv[:, jh:N_J],
                                            outv[:, jh:N_J], Alu.min)
                else:
                    nc.vector.tensor_tensor(outv, tmpv, outv, Alu.min)

        d2tv = d2t_d.rearrange("(j p) b h -> p j (b h)", p=P)
        accv = ACC[:].rearrange("p (j c) -> p j c", j=N_J)
        jh = N_J // 2
        nc.sync.dma_start(d2tv[:, 0:jh], accv[:, 0:jh])
        nc.scalar.dma_start(d2tv[:, jh:N_J], accv[:, jh:N_J])


HB = 8                    # input halo rows per side (>= R)
HE = HS + 2 * HB          # 80 extended rows per core
N_T3 = (B * HE) // P      # 10 scan tiles


def _k3_body(tc, out_d, xs_d):
    """Fused single-launch EDT: W-scans + transpose/square + banded H-pass +
    banded B-pass + sqrt.  Needs only an input halo of HB >= R rows (host
    pads with foreground), so no cross-core communication at all.

    xs_d:  [16, HE, 512] f32 dram (ExternalInput, host-padded h-slab)
    out_d: [512, 16, HS] f32 dram (ExternalOutput), distances, w-major
    """
    import concourse.mybir as mybir

    nc = tc.nc
    f16 = mybir.dt.float16
    f32 = mybir.dt.float32
    Alu = mybir.AluOpType
    Act = mybir.ActivationFunctionType
    N_J = W // P              # 4 w-groups
    CE = B * HE               # 1280 lines, also transposed free size per j
    C = B * HS                # 1024 interior (b,h) elements per w

    from concourse.masks import make_identity

    with tc.tile_pool(name="const", bufs=1) as cpool, \
         tc.tile_pool(name="big", bufs=1) as bpool, \
         tc.tile_pool(name="tmp", bufs=4) as tpool, \
         tc.tile_pool(name="psum", bufs=6, space="PSUM") as ppool, \
         tc.tile_pool(name="psumw", bufs=1, space="PSUM") as ppoolw:

        ident = cpool.tile([P, P], f16)
        make_identity(nc, ident[:])
        ones = cpool.tile([P, W], f16)
        nc.vector.memset(ones[:], 1.0)
        psw = ppoolw.tile([P, P], f16)
        nc.tensor.transpose(psw[:], ident[:], ident[:])

        XH = bpool.tile([P, N_T3 * W], f16)
        AALL = bpool.tile([P, N_T3 * W], f16)
        FALL = bpool.tile([P, N_T3 * W], f16)
        DALL = bpool.tile([P, N_T3 * W], f16)

        xflat = xs_d.rearrange("b h w -> (b h) w")
        for m in range(5):
            nc.gpsimd.dma_start(
                XH[:, 2 * W * m: 2 * W * (m + 1)].rearrange(
                    "p (g w) -> p g w", g=2),
                xflat[256 * m: 256 * (m + 1)].rearrange(
                    "(g pp) w -> pp g w", g=2))
        for m in range(5):
            nc.vector.tensor_scalar(
                AALL[:, 2 * W * m: 2 * W * (m + 1)],
                XH[:, 2 * W * m: 2 * W * (m + 1)],
                0.0, CLAMP, Alu.not_equal, Alu.mult)

        def seg(t):
            return W * t

        for t in range(N_T3):
            fa = FALL[:, W * t: W * (t + 1)]
            nc.vector.tensor_tensor_scan(
                fa, ones[:, 0:W], AALL[:, W * t: W * (t + 1)], CLAMP,
                Alu.add, Alu.min)
            nc.vector.tensor_tensor_scan(
                DALL[:, W * t: W * (t + 1)][:, ::-1], ones[:, 0:W],
                fa[:, ::-1], CLAMP, Alu.add, Alu.min)

        SQ = bpool.tile([P, N_J * CE], f16)     # w lines x (j, b, h80)
        for j in range(N_J):
            # two PSUM tiles per j: 640 fp16 = 1280B stays inside one 2KB
            # PSUM bank (a straddling AP faults the exec unit)
            for hf in range(2):
                ps = ppool.tile([P, CE // 2], f16, tag="ps")
                for tt_ in range(N_T3 // 2):
                    t = hf * (N_T3 // 2) + tt_
                    nc.tensor.transpose(
                        ps[:, P * tt_:P * (tt_ + 1)],
                        DALL[:, seg(t) + P * j: seg(t) + P * (j + 1)],
                        ident[:])
                nc.scalar.activation(
                    SQ[:, CE * j + (CE // 2) * hf:
                       CE * j + (CE // 2) * (hf + 1)], ps[:], Act.Square)

        # SQB[c] = SQ[c+1]: 4B-aligned source for odd H-shifts
        SQB = bpool.tile([P, N_J * CE], f16)
        nc.scalar.activation(SQB[:, 0:N_J * CE - 1], SQ[:, 1:N_J * CE],
                             Act.Copy, bias=0.0)

        # banded pass H: pair = min(left, right), tmp = pair + s^2,
        # accH = min(accH, tmp).  Output interior h in [HB, HB+HS).
        ACH = bpool.tile([P, N_J * C], f16)
        sq5 = SQ[:].rearrange("p (j b h) -> p j b h", j=N_J, b=B)
        sqb5 = SQB[:].rearrange("p (j b h) -> p j b h", j=N_J, b=B)
        ah4 = ACH[:].rearrange("p (j b h) -> p j b h", j=N_J, b=B)
        center = sq5[:, :, :, HB:HB + HS]
        order = sorted(range(1, R + 1), key=lambda s: (s % 2, -s))
        for si, s in enumerate(order):
            if s % 2 == 0:
                lo = sq5[:, :, :, HB - s:HB - s + HS]
                hi = sq5[:, :, :, HB + s:HB + s + HS]
            else:
                lo = sqb5[:, :, :, HB - s - 1:HB - s - 1 + HS]
                hi = sqb5[:, :, :, HB + s - 1:HB + s - 1 + HS]
            pair = tpool.tile([P, N_J * C], f16, tag="pair")
            pair4 = pair[:].rearrange("p (j b h) -> p j b h", j=N_J, b=B)
            if si == 0:
                jh = N_J // 2
                nc.vector.tensor_tensor(pair4[:, 0:jh], lo[:, 0:jh],
                                        hi[:, 0:jh], Alu.min)
                nc.vector.tensor_tensor(pair4[:, jh:N_J], lo[:, jh:N_J],
                                        hi[:, jh:N_J], Alu.min)
            else:
                nc.vector.tensor_tensor(pair4, lo, hi, Alu.min)
            tmp = tpool.tile([P, N_J * C], f16, tag="tmp")
            tmp4 = tmp[:].rearrange("p (j b h) -> p j b h", j=N_J, b=B)
            if s == 2:
                nc.vector.tensor_scalar(tmp4, pair4, float(s * s), None,
                                        Alu.add)
            else:
                nc.scalar.activation(tmp4, pair4, Act.Copy, bias=float(s * s))
            if si == 0:
                nc.vector.tensor_tensor(ah4, tmp4, center, Alu.min)
            else:
                nc.vector.tensor_tensor(ah4, tmp4, ah4, Alu.min)

        # banded pass B on the compact (j, b, h64) layout
        ACC = bpool.tile([P, N_J * C], f16)
        ac4 = ACC[:].rearrange("p (j b h) -> p j b h", j=N_J, b=B)
        nc.scalar.activation(ac4[:, :, B - 1:B, :], ah4[:, :, B - 1:B, :],
                             Act.Copy, bias=0.0)
        first = True
        for s in range(1, R + 1):
            bc = B - s
            for sgn in (1, -1):
                if sgn > 0:
                    srcv = ah4[:, :, s:s + bc, :]
                    outv = ac4[:, :, 0:bc, :]
                else:
                    srcv = ah4[:, :, 0:bc, :]
                    outv = ac4[:, :, s:B, :]
                tmp = tpool.tile([P, N_J * C], f16, tag="tmpb")
                tmpv = tmp[:].rearrange(
                    "p (j b h) -> p j b h", j=N_J, b=B)[:, :, 0:bc, :]
                if s >= 2:
                    nc.scalar.activation(tmpv, srcv, Act.Copy,
                                         bias=float(s * s))
                else:
                    nc.vector.tensor_scalar(tmpv, srcv, float(s * s), None,
                                            Alu.add)
                if first:
                    nc.vector.tensor_tensor(outv, tmpv, ah4[:, :, 0:bc, :],
                                            Alu.min)
                    first = False
                elif s == R and sgn == -1:
                    for j in range(N_J):
                        nc.vector.tensor_tensor(
                            outv[:, j:j + 1], tmpv[:, j:j + 1],
                            outv[:, j:j + 1], Alu.min)
                else:
                    nc.vector.tensor_tensor(outv, tmpv, outv, Alu.min)

        OUT = bpool.tile([P, N_J * C], f32)
        outd = out_d.rearrange("(j p) b h -> p j (b h)", p=P)
        outs = OUT[:].rearrange("p (j c) -> p j c", j=N_J)
        accs = ACC[:].rearrange("p (j c) -> p j c", j=N_J)
        for j in range(N_J):
            nc.scalar.activation(outs[:, j:j + 1], accs[:, j:j + 1], Act.Sqrt)
            eng = nc.sync if j % 2 == 0 else nc.scalar
            eng.dma_start(outd[:, j:j + 1], outs[:, j:j + 1])


def _k2_body(tc, out_d, d2s_d):
    """Banded pass H + sqrt.

    d2s_d: [WS, 16, 512] f16 dram (ExternalInput)
    out_d: [WS, 16, 512] f32 dram (ExternalOutput), distances

    Row-padded layout (PADH pad columns, value 2047 == +inf) so every
    shifted operand is full-width; odd shifts read SRCB (CALL displaced by
    one column) so all hot operands stay 4B-aligned.  Per |s| the two
    directions fold as pair = min(left, right) (DVE), tmp = pair + s^2
    (ACT, or DVE tensor_scalar for one shift), acc = min(acc, tmp) (DVE).
    """
    import concourse.mybir as mybir

    nc = tc.nc
    f16 = mybir.dt.float16
    f32 = mybir.dt.float32
    Alu = mybir.AluOpType
    Act = mybir.ActivationFunctionType
    N_G = (WS * B) // P       # 8 groups of 128 (w,b) lines
    PADH = 8
    HP = H + 2 * PADH         # 528 padded row width

    with tc.tile_pool(name="big", bufs=1) as bpool, \
         tc.tile_pool(name="tmp", bufs=3) as tpool:

        CALL = bpool.tile([P, N_G * HP], f16)
        SRCB = bpool.tile([P, N_G * HP], f16)
        ACC = bpool.tile([P, N_G * HP], f16)
        # pad strips (+inf) for CALL rows [0:PADH) and [PADH+H:HP),
        # for SRCB rows [0:PADH-1) and [PADH-1+H:HP)
        cstrips = CALL[:].rearrange("p (g h) -> p g h", g=N_G)
        nc.gpsimd.memset(cstrips[:, :, 0:PADH], 2047.0)
        nc.gpsimd.memset(cstrips[:, :, PADH + H:HP], 2047.0)
        strips = SRCB[:].rearrange("p (g h) -> p g h", g=N_G)
        nc.gpsimd.memset(strips[:, :, 0:PADH - 1], 2047.0)
        nc.gpsimd.memset(strips[:, :, PADH - 1 + H:HP], 2047.0)

        call3 = CALL[:].rearrange("p (g h) -> p g h", g=N_G)
        srcb3 = SRCB[:].rearrange("p (g h) -> p g h", g=N_G)
        acc3 = ACC[:].rearrange("p (g h) -> p g h", g=N_G)
        accint = acc3[:, :, PADH:PADH + H]
        d2sv = d2s_d.rearrange("(g ww) b h -> (ww b) g (h)",
                               g=N_G, ww=WS // N_G)
        # three copies of the input land by DMA: source, 1-column-shifted
        # source (odd-shift alignment helper), and the accumulator init.
        nc.sync.dma_start(call3[:, :, PADH:PADH + H], d2sv)
        nc.scalar.dma_start(srcb3[:, :, PADH - 1:PADH - 1 + H], d2sv)
        nc.sync.dma_start(accint, d2sv)

        order = sorted(range(1, R + 1), key=lambda s: (s % 2, -s))
        assert order[-1] == 1
        for s in order:
            if s % 2 == 0:
                lo = call3[:, :, PADH - s:PADH - s + H]
                hi = call3[:, :, PADH + s:PADH + s + H]
            else:
                lo = srcb3[:, :, PADH - s - 1:PADH - s - 1 + H]
                hi = srcb3[:, :, PADH + s - 1:PADH + s - 1 + H]
            pair = tpool.tile([P, N_G * H], f16, tag="pair")
            pair3 = pair[:].rearrange("p (g h) -> p g h", g=N_G)
            nc.vector.tensor_tensor(pair3, lo, hi, Alu.min)
            tmp = tpool.tile([P, N_G * H], f16, tag="tmp")
            tmp3 = tmp[:].rearrange("p (g h) -> p g h", g=N_G)
            if s == 2:
                nc.vector.tensor_scalar(tmp3, pair3, float(s * s), None,
                                        Alu.add)
            else:
                nc.scalar.activation(tmp3, pair3, Act.Copy, bias=float(s * s))
            if s == 1:
                # split the chain tail so sqrt + store of the first half
                # overlap the second half's min
                hh = N_G // 2
                nc.vector.tensor_tensor(accint[:, 0:hh], tmp3[:, 0:hh],
                                        accint[:, 0:hh], Alu.min)
                nc.vector.tensor_tensor(accint[:, hh:N_G], tmp3[:, hh:N_G],
                                        accint[:, hh:N_G], Alu.min)
            else:
                nc.vector.tensor_tensor(accint, tmp3, accint, Alu.min)

        OUT = bpool.tile([P, N_G * H], f32)
        out3 = OUT[:].rearrange("p (g h) -> p g h", g=N_G)
        outv = out_d.rearrange("(g ww) b h -> (ww b) g (h)",
                               g=N_G, ww=WS // N_G)
        hh = N_G // 2
        for half in range(2):
            sl = slice(hh * half, hh * (half + 1))
            nc.scalar.activation(out3[:, sl], accint[:, sl], Act.Sqrt)
            eng = nc.sync if half == 0 else nc.scalar
            eng.dma_start(outv[:, sl], out3[:, sl])


def _split_multi_waits(nc):
    """Walrus in this toolchain encodes at most ONE sync wait per hardware
    instruction.  Hoist extra waits onto same-engine NoOp carriers inserted
    immediately before the over-subscribed instruction (program order on the
    engine preserves the semantics exactly)."""
    import concourse.mybir as mybir

    n = 0
    for fn in nc.m.functions:
        for blk in fn.blocks:
            insts = blk.instructions
            out = []
            for inst in insts:
                si = inst.sync_info
                if si is not None and len(si.on_wait) > 1:
                    waits = list(si.on_wait)
                    for w in waits[:-1]:
                        nop = mybir.InstNoOp(
                            name=f"waitsplit-{n}", ins=[], outs=[])
                        n += 1
                        nop.engine = inst.engine
                        nop.sync_info = mybir.SyncInfo(
                            on_wait=[w], on_update=[])
                        out.append(nop)
                    inst.sync_info = mybir.SyncInfo(
                        on_wait=[waits[-1]], on_update=list(si.on_update))
                out.append(inst)
            blk.instructions = out
    return n


def _make_tc_class():
    """TileContext whose kernel-tail drain is split into one drain per proc.

    The stock tail emits a single sync-engine Drain waiting on every
    outstanding processor; this walrus build only encodes ONE sync wait per
    instruction, so the aggregated drain fails codegen.  Semantics are
    identical — the waits just land on consecutive Drain instructions.
    """
    import concourse.tile as tile
    from concourse.vector_clock import ScopedClock, VectorClock

    class SplitDrainTileContext(tile.TileContext):
        def _drain_and_barrier(self, tick_clock, wait_clock):
            gvc = tick_clock.global_clock
            for proc in range(len(gvc)):
                t = gvc[proc]
                if t <= 0:
                    continue
                d = self.nc.sync.drain()
                sv = VectorClock([0] * len(gvc))
                sv.require_at_least(proc, t)
                wait_clock.add_sem_waits(d.ins, ScopedClock({None: sv}))
            self.nc.all_engine_barrier()
            assert self.sems is not None
            popped = self.nc._tile_sem_poison_stack.pop()
            assert popped is self._sem_poison
            self.nc.clear_and_free_semaphores(
                list(self.sems.allocated().values()))
            self.nc.all_engine_barrier()

    return SplitDrainTileContext


def _build():
    """Build the fused Bass module (done once per process)."""
    import concourse.bass as bass
    import concourse.mybir as mybir

    f16 = mybir.dt.float16
    f32 = mybir.dt.float32
    TC = _make_tc_class()

    nc3 = bass.Bass("TRN2", debug=False, num_devices=NCORES)
    xs3_d = nc3.dram_tensor("xs3", [B, HE, W], f32,
                            kind="ExternalInput").ap()
    ot3_d = nc3.dram_tensor("ot3", [W, B, HS], f32,
                            kind="ExternalOutput").ap()
    with TC(nc3) as tc:
        _k3_body(tc, ot3_d, xs3_d)
    _split_multi_waits(nc3)
    return (nc3,)


def _build_two_launch():
    """Older two-launch pipeline (kept as reference/fallback)."""
    import concourse.bass as bass
    import concourse.mybir as mybir

    f16 = mybir.dt.float16
    f32 = mybir.dt.float32
    TC = _make_tc_class()

    nc1 = bass.Bass("TRN2", debug=False, num_devices=NCORES)
    xs_d = nc1.dram_tensor("xs", [B, HS, W], f32, kind="ExternalInput").ap()
    d2t_d = nc1.dram_tensor("d2t", [W, B, HS], f16, kind="ExternalOutput").ap()
    with TC(nc1) as tc:
        _k1_body(tc, d2t_d, xs_d)
    _split_multi_waits(nc1)

    nc2 = bass.Bass("TRN2", debug=False, num_devices=NCORES)
    d2s_d = nc2.dram_tensor("d2s", [WS, B, H], f16, kind="ExternalInput").ap()
    out_d = nc2.dram_tensor("ot", [WS, B, H], f32, kind="ExternalOutput").ap()
    with TC(nc2) as tc:
        _k2_body(tc, out_d, d2s_d)
    _split_multi_waits(nc2)

    return nc1, nc2


def _host_exact_edt(x):
    """Exact host fallback: banded numpy EDT with growing radius (f32)."""
    INF = np.float32(1e9)
    r = 2 * R
    while True:
        d0 = np.where(x != 0, INF, np.float32(0.0))
        fwd = np.empty_like(d0)
        st = np.full(d0.shape[:2], INF, np.float32)
        for w in range(W):
            st = np.minimum(st + 1.0, d0[:, :, w]); fwd[:, :, w] = st
        st = np.full(d0.shape[:2], INF, np.float32)
        bwd = np.empty_like(d0)
        for w in range(W - 1, -1, -1):
            st = np.minimum(st + 1.0, d0[:, :, w]); bwd[:, :, w] = st
        d2 = np.minimum(fwd, bwd) ** 2
        for axis in (0, 1):
            src = d2
            acc = src.copy()
            rr = min(r, x.shape[axis] - 1)
            for s in range(1, rr + 1):
                sl_lo = [slice(None)] * 3
                sl_hi = [slice(None)] * 3
                sl_lo[axis] = slice(0, x.shape[axis] - s)
                sl_hi[axis] = slice(s, None)
                np.minimum(acc[tuple(sl_lo)], src[tuple(sl_hi)] + s * s,
                           out=acc[tuple(sl_lo)])
                np.minimum(acc[tuple(sl_hi)], src[tuple(sl_lo)] + s * s,
                           out=acc[tuple(sl_hi)])
            d2 = acc
        out = np.sqrt(d2)
        # exact when every per-axis offset fits in the band; r >= max dim
        # means the bands are complete regardless of the value of out
        if out.max() <= r or r >= max(x.shape):
            return out.astype(np.float32)
        r *= 2


_RUNNER = None


def _make_runner(nc, n_cores):
    """Build the sharded PJRT callable once (run_bass_kernel_spmd re-traces
    and re-jits on every call; caching saves ~1 s per kernel() invocation)."""
    import jax
    import numpy as _np
    from jax.sharding import Mesh, PartitionSpec
    from jax.experimental.shard_map import shard_map
    import concourse.mybir as mybir
    from concourse import bass2jax

    bass2jax.install_neuronx_cc_hook()
    partition_name = (nc.partition_id_tensor.name
                      if nc.partition_id_tensor else None)
    in_names, out_names, out_avals, zero_outs = [], [], [], []
    for alloc in nc.m.functions[0].allocations:
        if not isinstance(alloc, mybir.MemoryLocationSet):
            continue
        name = alloc.memorylocations[0].name
        if alloc.kind == "ExternalInput":
            if name != partition_name:
                in_names.append(name)
        elif alloc.kind == "ExternalOutput":
            out_avals.append(jax.core.ShapedArray(
                tuple(alloc.tensor_shape), mybir.dt.np(alloc.dtype)))
            out_names.append(name)
            zero_outs.append(_np.zeros(tuple(alloc.tensor_shape),
                                       mybir.dt.np(alloc.dtype)))
    all_in = list(in_names) + list(out_names)
    if partition_name is not None:
        all_in.append(partition_name)

    def _body(*args):
        operands = list(args)
        if partition_name is not None:
            operands.append(bass2jax.partition_id_tensor())
        return tuple(bass2jax._bass_exec_p.bind(
            *operands, out_avals=tuple(out_avals), in_names=tuple(all_in),
            out_names=tuple(out_names), lowering_input_output_aliases=(),
            sim_require_finite=True, sim_require_nnan=True, nc=nc))

    devices = jax.devices()[:n_cores]
    mesh = Mesh(_np.asarray(devices), ("core",))
    n_io = len(in_names) + len(out_names)
    fn = jax.jit(shard_map(_body, mesh=mesh,
                           in_specs=(PartitionSpec("core"),) * n_io,
                           out_specs=(PartitionSpec("core"),) * len(out_names),
                           check_rep=False), keep_unused=True)

    def run(in_maps):
        concat_in = [_np.concatenate([_np.asarray(in_maps[c][n])
                                      for c in range(n_cores)], axis=0)
                     for n in in_names]
        concat_zero = [_np.zeros((n_cores * z.shape[0], *z.shape[1:]), z.dtype)
                       for z in zero_outs]
        outs = fn(*concat_in, *concat_zero)
        return [{name: _np.asarray(outs[i]).reshape(
                    n_cores, *out_avals[i].shape)[c]
                 for i, name in enumerate(out_names)}
                for c in range(n_cores)]

    return run


def kernel(x):
    global _BUILT, _RUNNER
    x = np.asarray(x)
    assert x.shape == (B, H, W)
    if x.dtype != np.float32:
        x = x.astype(np.float32)

    if _BUILT is None:
        _BUILT = _build()
    (nc3,) = _BUILT
    if _RUNNER is None:
        _RUNNER = _make_runner(nc3, NCORES)
    LAST_RESULTS.clear()

    xp = np.pad(x, ((0, 0), (HB, HB), (0, 0)), constant_values=1.0)
    in3 = [{"xs3": np.ascontiguousarray(xp[:, k * HS:k * HS + HE, :])}
           for k in range(NCORES)]
    results = _RUNNER(in3)
    outt = np.concatenate([results[k]["ot3"] for k in range(NCORES)], axis=2)

    out = outt.transpose(1, 2, 0)          # (w,b,h) -> (b,h,w)

    # Banding is exact iff the true max distance < R + 1 (per-axis integer
    # offsets of the optimal zero are bounded by floor of the distance, and
    # the banded result upper-bounds the true one).
    if float(np.max(out)) >= R + 1:
        out = _host_exact_edt(x)

    nan_mask = np.isnan(x)
    if nan_mask.any():
        out = np.where(nan_mask, np.float32(np.nan), out)
    return out



# revision 10
# speedup vs baseline: 1.8892x; 1.8892x over previous
"""Trainium2 Bass kernel: exact 3D Euclidean distance transform of a binary
(16, 512, 512) float32 volume — distance from every nonzero voxel to the
nearest zero voxel over ALL three axes (batch participates in the metric),
matching scipy.ndimage.distance_transform_edt on the full array.

Fast path / slow path split:
  Device (this kernel): separable EDT with an exact W pass (fwd/bwd
  saturating scans) and parabola min-plus passes along H and B banded at
  radius R=2.  This is exact for every voxel whose true distance is < R+1
  (its optimal per-axis offsets are <= floor(d) <= R), i.e. for ~99% of
  voxels at the 5%-background density this module targets.
  Host: every voxel with device d^2 >= (R+1)^2 (any voxel the band could
  have gotten wrong necessarily lands in this set, because the banded value
  only ever over-estimates and a band violation implies true d >= R+1) is
  re-solved exactly by a vectorized radius-6 window search; if any such
  voxel has no zero within distance < 6 the whole volume falls back to an
  exact host EDT.  The patched result is exact everywhere, for any input.

Device pipeline (values are small integers <= CLAMP^2+8, exact in fp16,
which unlocks the DVE 2x/4x perf modes):
  pass W: 1D nearest-zero distance along W via fwd/bwd scans
          (tensor_tensor_scan, DVE-only op), squared during the PSUM
          evacuation of a PE transpose (ACT Square).
  pass H: banded parabola min-plus along H (radius 2).
  pass B: banded parabola min-plus along B (radius 2).
  Output is d^2 in fp16, w-major; the host does the final sqrt.

Engine split (only DVE and ACT can do general elementwise work on TRN2
silicon; Pool rejects TensorTensor/TensorScalarPtr at codegen):
  DVE: scans (1x), every min (tensor_tensor, 2x), first binarize chunk and
       the B-pass +1 adds (tensor_scalar, 4x).
  ACT: binarize Relu(CLAMP*x), PSUM evacuation fused with Square, +s^2 adds
       (Copy + bias).
  PE:  transposes.  SP(sync): input DMA issue.  Pool: constants only.
The H pass is split into (b-chunk x j-half) pieces whose b-chunks only
depend on already-evacuated thirds of the squared field; pairs are emitted
before folds so the serial fold chain never starves; the B pass runs per
j-half / per j so each output DMA starts as soon as its slice is final.

Sharding: data-parallel over H (8 slabs of 64 rows); the W-scan needs full
W and the B-pass full B, which each slab has; the H-pass needs a 4-row
input halo (host pads with foreground).  No cross-core communication.
I/O is fp16: binary input survives the cast exactly; d^2 outputs are small
exact integers.

Hardware quirk: several instruction encodings accept only ONE semaphore
wait; _split_multi_waits hoists extra waits onto same-engine NoOp carriers.
"""
import numpy as np

B, H, W = 16, 512, 512
NCORES = 8
HS = H // NCORES          # 64 interior rows per core
P = 128
CLAMP = 32.0
R = 2                     # band radius of the H and B passes

HB = 4                    # input halo rows per side (>= R, tile-aligning)
HE = HS + 2 * HB          # 72 extended rows per core
N_T = (B * HE) // P       # 9 scan tiles
N_J = W // P              # 4 w-groups
CE = B * HE               # 1152 transposed lines per j-group
C = B * HS                # 1024 interior (b,h) elements per j-group
NG = 3                    # PSUM evacuation groups per j (3 tiles each)

_BUILT = None
LAST_RESULTS = []   # kept for the test harness's profiling hook


def _k5_body(tc, out_d, xs_d):
    """Fused single-launch banded-EDT device pass.

    xs_d:  [16, HE, 512] f16 dram (ExternalInput, host-padded h-slab)
    out_d: [512, 16, HS] f16 dram (ExternalOutput), squared distances,
           w-major
    """
    import concourse.mybir as mybir

    nc = tc.nc
    f16 = mybir.dt.float16
    Alu = mybir.AluOpType
    Act = mybir.ActivationFunctionType

    from concourse.masks import make_identity

    with tc.tile_pool(name="const", bufs=1) as cpool, \
         tc.tile_pool(name="big", bufs=1) as bpool, \
         tc.tile_pool(name="htmp", bufs=5) as hpool, \
         tc.tile_pool(name="btmp", bufs=6) as tbpool, \
         tc.tile_pool(name="psum", bufs=4, space="PSUM") as ppool, \
         tc.tile_pool(name="psumw", bufs=1, space="PSUM") as ppoolw:

        ones = cpool.tile([P, W], f16)
        nc.gpsimd.memset(ones[:], 1.0)
        ident = cpool.tile([P, P], f16)
        make_identity(nc, ident[:])
        # dummy transpose so PE observes the gpsimd-built identity before the
        # real transposes (keeps every matmul at <= 1 semaphore wait)
        psw = ppoolw.tile([P, P], f16)
        nc.tensor.transpose(psw[:], ident[:], ident[:])

        XH = bpool.tile([P, N_T * W], f16)      # raw x, f16
        AALL = bpool.tile([P, N_T * W], f16)    # d0 = (x != 0) * CLAMP
        FALL = bpool.tile([P, N_T * W], f16)    # fwd scan
        DALL = bpool.tile([P, N_T * W], f16)    # bwd scan of fwd = 1D dist

        # input DMAs on the sync queue (its trigger issue starts immediately;
        # the Pool queue is busy building the identity).  First chunk is one
        # tile so the first binarize + scan start as early as possible.
        xflat = xs_d.rearrange("b h w -> (b h) w")
        chunks = [(0, 1), (1, 2), (3, 2), (5, 2), (7, 2)]
        for t0, k in chunks:
            if k == 1:
                nc.sync.dma_start(XH[:, W * t0: W * (t0 + 1)],
                                  xflat[P * t0: P * (t0 + 1)])
            else:
                nc.sync.dma_start(
                    XH[:, W * t0: W * (t0 + k)].rearrange(
                        "p (g w) -> p g w", g=k),
                    xflat[P * t0: P * (t0 + k)].rearrange(
                        "(g pp) w -> pp g w", g=k))
        # binarize: first tile on DVE (4x tensor_scalar, shortest latency to
        # the first scan), the rest on ACT (Relu(CLAMP*x) -> {0, CLAMP}).
        nc.vector.tensor_scalar(AALL[:, 0:W], XH[:, 0:W], 0.0, CLAMP,
                                Alu.not_equal, Alu.mult)
        for t0, k in chunks[1:]:
            nc.scalar.activation(AALL[:, W * t0: W * (t0 + k)],
                                 XH[:, W * t0: W * (t0 + k)],
                                 Act.Relu, scale=CLAMP)

        for t in range(N_T):
            fa = FALL[:, W * t: W * (t + 1)]
            nc.vector.tensor_tensor_scan(
                fa, ones[:, 0:W], AALL[:, W * t: W * (t + 1)], CLAMP,
                Alu.add, Alu.min)
            nc.vector.tensor_tensor_scan(
                DALL[:, W * t: W * (t + 1)][:, ::-1], ones[:, 0:W],
                fa[:, ::-1], CLAMP, Alu.add, Alu.min)

        # transpose + evacuate-with-Square, in NG groups of 3 scan tiles per
        # j-group so the H pass can start on low-b chunks early.
        SQ = bpool.tile([P, N_J * CE], f16)     # w lines x (j, b, h72)
        GT = N_T // NG                          # 3 tiles per group
        for g in range(NG):
            for j in range(N_J):
                ps = ppool.tile([P, GT * P], f16, tag="ps")
                for tt in range(GT):
                    t = g * GT + tt
                    nc.tensor.transpose(
                        ps[:, P * tt: P * (tt + 1)],
                        DALL[:, W * t + P * j: W * t + P * (j + 1)],
                        ident[:])
                nc.scalar.activation(
                    SQ[:, CE * j + GT * P * g: CE * j + GT * P * (g + 1)],
                    ps[:], Act.Square)

        sq5 = SQ[:].rearrange("p (j b h) -> p j b h", j=N_J, b=B)
        ACH = bpool.tile([P, N_J * C], f16)
        ah4 = ACH[:].rearrange("p (j b h) -> p j b h", j=N_J, b=B)
        ACC = bpool.tile([P, N_J * C], f16)
        ac4 = ACC[:].rearrange("p (j b h) -> p j b h", j=N_J, b=B)

        # b-chunks: (0:5) lies inside evacuation group 0 (lines < 360 < 384),
        # (5:10) inside groups 0-1, (10:16) needs all three.
        BCH = [(0, 5), (5, 10), (10, 16)]

        def h_chunk(bc, jh):
            """Pass H on one (b-chunk, j-half): pairs (DVE), +s^2 in place
            (ACT), fold chain (DVE).  Pairs are all emitted before the folds
            so DVE never blocks on an ACT add."""
            b0, b1 = BCH[bc]
            nb = b1 - b0
            sq = sq5[:, 2 * jh:2 * (jh + 1), b0:b1, :]
            ts = []
            for s in range(1, R + 1):
                lo = sq[:, :, :, HB - s:HB - s + HS]
                hi = sq[:, :, :, HB + s:HB + s + HS]
                t_ = hpool.tile([P, 2 * nb * HS], f16, tag="hq")
                tv = t_[:].rearrange("p (j b h) -> p j b h", j=2, b=nb)
                nc.vector.tensor_tensor(tv, lo, hi, Alu.min)
                ts.append(tv)
            for s in range(1, R + 1):
                nc.scalar.activation(ts[s - 1], ts[s - 1], Act.Copy,
                                     bias=float(s * s))
            a = ah4[:, 2 * jh:2 * (jh + 1), b0:b1, :]
            ctr = sq[:, :, :, HB:HB + HS]
            nc.vector.tensor_tensor(a, ts[0], ctr, Alu.min)
            for s in range(2, R + 1):
                nc.vector.tensor_tensor(a, ts[s - 1], a, Alu.min)

        def b_adds(j0, nj):
            """ACT-side prep for pass B on j-groups [j0, j0+nj): the b = B-1
            strip of the accumulator (its center term) and the shared +s^2
            tensors for s >= 2 (s = 1 is a DVE 4x tensor_scalar in
            b_folds)."""
            a = ah4[:, j0:j0 + nj]
            c = ac4[:, j0:j0 + nj]
            nc.scalar.activation(c[:, :, B - 1:B, :], a[:, :, B - 1:B, :],
                                 Act.Copy, bias=0.0)
            tbs = []
            for s in range(2, R + 1):
                tb = tbpool.tile([P, nj * C], f16, tag=f"tb{nj}")
                tv = tb[:].rearrange("p (j b h) -> p j b h", j=nj, b=B)
                nc.scalar.activation(tv, a, Act.Copy, bias=float(s * s))
                tbs.append(tv)
            return tbs

        def b_folds(j0, nj, tbs):
            """Pass B directional folds on j-groups [j0, j0+nj) (DVE).  The
            s=1 add runs on DVE (4x) so the chain starts without ACT."""
            a = ah4[:, j0:j0 + nj]
            c = ac4[:, j0:j0 + nj]
            tb1 = tbpool.tile([P, nj * C], f16, tag=f"tb1{nj}")
            t1 = tb1[:].rearrange("p (j b h) -> p j b h", j=nj, b=B)
            nc.vector.tensor_scalar(t1, a, 1.0, None, Alu.add)
            for s in range(1, R + 1):
                tv = t1 if s == 1 else tbs[s - 2]
                bc = B - s
                if s == 1:
                    nc.vector.tensor_tensor(c[:, :, 0:bc, :],
                                            tv[:, :, s:B, :],
                                            a[:, :, 0:bc, :], Alu.min)
                else:
                    nc.vector.tensor_tensor(c[:, :, 0:bc, :],
                                            tv[:, :, s:B, :],
                                            c[:, :, 0:bc, :], Alu.min)
                nc.vector.tensor_tensor(c[:, :, s:B, :], tv[:, :, 0:bc, :],
                                        c[:, :, s:B, :], Alu.min)

        outd = out_d.rearrange("(j p) b h -> p j (b h)", p=P)
        accs = ACC[:].rearrange("p (j c) -> p j c", j=N_J)

        # DVE order keeps the engine stall-free: all jh0 H chunks, one jh1
        # chunk (gives ACT time for the jh0 B adds), B(jh0) + its DMA, the
        # remaining jh1 chunks, then per-j B chains each followed by its DMA.
        h_chunk(0, 0)
        h_chunk(1, 0)
        h_chunk(2, 0)
        tbs0 = b_adds(0, 2)
        h_chunk(0, 1)
        b_folds(0, 2, tbs0)
        nc.scalar.dma_start(outd[:, 0:2], accs[:, 0:2])
        h_chunk(1, 1)
        h_chunk(2, 1)
        tbs2 = b_adds(2, 1)
        b_folds(2, 1, tbs2)
        nc.sync.dma_start(outd[:, 2:3], accs[:, 2:3])
        tbs3 = b_adds(3, 1)
        b_folds(3, 1, tbs3)
        nc.gpsimd.dma_start(outd[:, 3:4], accs[:, 3:4])


def _split_multi_waits(nc):
    """Walrus in this toolchain encodes at most ONE sync wait per hardware
    instruction.  Hoist extra waits onto same-engine NoOp carriers inserted
    immediately before the over-subscribed instruction (program order on the
    engine preserves the semantics exactly)."""
    import concourse.mybir as mybir

    n = 0
    for fn in nc.m.functions:
        for blk in fn.blocks:
            insts = blk.instructions
            out = []
            for inst in insts:
                si = inst.sync_info
                if si is not None and len(si.on_wait) > 1:
                    waits = list(si.on_wait)
                    for w in waits[:-1]:
                        nop = mybir.InstNoOp(
                            name=f"waitsplit-{n}", ins=[], outs=[])
                        n += 1
                        nop.engine = inst.engine
                        nop.sync_info = mybir.SyncInfo(
                            on_wait=[w], on_update=[])
                        out.append(nop)
                    inst.sync_info = mybir.SyncInfo(
                        on_wait=[waits[-1]], on_update=list(si.on_update))
                out.append(inst)
            blk.instructions = out
    return n


def _make_tc_class():
    """TileContext whose kernel-tail drain is split into one drain per proc.

    The stock tail emits a single sync-engine Drain waiting on every
    outstanding processor; this walrus build only encodes ONE sync wait per
    instruction, so the aggregated drain fails codegen.  Semantics are
    identical — the waits just land on consecutive Drain instructions.
    """
    import concourse.tile as tile
    from concourse.vector_clock import ScopedClock, VectorClock

    class SplitDrainTileContext(tile.TileContext):
        def _drain_and_barrier(self, tick_clock, wait_clock):
            gvc = tick_clock.global_clock
            for proc in range(len(gvc)):
                t = gvc[proc]
                if t <= 0:
                    continue
                d = self.nc.sync.drain()
                sv = VectorClock([0] * len(gvc))
                sv.require_at_least(proc, t)
                wait_clock.add_sem_waits(d.ins, ScopedClock({None: sv}))
            self.nc.all_engine_barrier()
            assert self.sems is not None
            popped = self.nc._tile_sem_poison_stack.pop()
            assert popped is self._sem_poison
            self.nc.clear_and_free_semaphores(
                list(self.sems.allocated().values()))
            self.nc.all_engine_barrier()

    return SplitDrainTileContext


def _build():
    """Build the fused Bass module (done once per process)."""
    import concourse.bass as bass
    import concourse.mybir as mybir

    f16 = mybir.dt.float16
    TC = _make_tc_class()

    nc5 = bass.Bass("TRN2", debug=False, num_devices=NCORES)
    xs5_d = nc5.dram_tensor("xs5", [B, HE, W], f16,
                            kind="ExternalInput").ap()
    ot5_d = nc5.dram_tensor("ot5", [W, B, HS], f16,
                            kind="ExternalOutput").ap()
    with TC(nc5) as tc:
        _k5_body(tc, ot5_d, xs5_d)
    _split_multi_waits(nc5)
    return (nc5,)


def _host_exact_edt(x):
    """Exact host fallback: banded numpy EDT with growing radius (f32)."""
    INF = np.float32(1e9)
    r = 8
    while True:
        d0 = np.where(x != 0, INF, np.float32(0.0))
        fwd = np.empty_like(d0)
        st = np.full(d0.shape[:2], INF, np.float32)
        for w in range(W):
            st = np.minimum(st + 1.0, d0[:, :, w]); fwd[:, :, w] = st
        st = np.full(d0.shape[:2], INF, np.float32)
        bwd = np.empty_like(d0)
        for w in range(W - 1, -1, -1):
            st = np.minimum(st + 1.0, d0[:, :, w]); bwd[:, :, w] = st
        d2 = np.minimum(fwd, bwd) ** 2
        for axis in (0, 1):
            src = d2
            acc = src.copy()
            rr = min(r, x.shape[axis] - 1)
            for s in range(1, rr + 1):
                sl_lo = [slice(None)] * 3
                sl_hi = [slice(None)] * 3
                sl_lo[axis] = slice(0, x.shape[axis] - s)
                sl_hi[axis] = slice(s, None)
                np.minimum(acc[tuple(sl_lo)], src[tuple(sl_hi)] + s * s,
                           out=acc[tuple(sl_lo)])
                np.minimum(acc[tuple(sl_hi)], src[tuple(sl_lo)] + s * s,
                           out=acc[tuple(sl_hi)])
            d2 = acc
        out = np.sqrt(d2)
        # exact when every per-axis offset fits in the band; r >= max dim
        # means the bands are complete regardless of the value of out
        if out.max() <= r or r >= max(x.shape):
            return out.astype(np.float32)
        r *= 2


_RUNNER = None


def _make_runner(nc, n_cores):
    """Build the sharded PJRT callable once (run_bass_kernel_spmd re-traces
    and re-jits on every call; caching saves ~1 s per kernel() invocation)."""
    import jax
    import numpy as _np
    from jax.sharding import Mesh, PartitionSpec
    from jax.experimental.shard_map import shard_map
    import concourse.mybir as mybir
    from concourse import bass2jax

    bass2jax.install_neuronx_cc_hook()
    partition_name = (nc.partition_id_tensor.name
                      if nc.partition_id_tensor else None)
    in_names, out_names, out_avals, zero_outs = [], [], [], []
    for alloc in nc.m.functions[0].allocations:
        if not isinstance(alloc, mybir.MemoryLocationSet):
            continue
        name = alloc.memorylocations[0].name
        if alloc.kind == "ExternalInput":
            if name != partition_name:
                in_names.append(name)
        elif alloc.kind == "ExternalOutput":
            out_avals.append(jax.core.ShapedArray(
                tuple(alloc.tensor_shape), mybir.dt.np(alloc.dtype)))
            out_names.append(name)
            zero_outs.append(_np.zeros(tuple(alloc.tensor_shape),
                                       mybir.dt.np(alloc.dtype)))
    all_in = list(in_names) + list(out_names)
    if partition_name is not None:
        all_in.append(partition_name)

    def _body(*args):
        operands = list(args)
        if partition_name is not None:
            operands.append(bass2jax.partition_id_tensor())
        return tuple(bass2jax._bass_exec_p.bind(
            *operands, out_avals=tuple(out_avals), in_names=tuple(all_in),
            out_names=tuple(out_names), lowering_input_output_aliases=(),
            sim_require_finite=True, sim_require_nnan=True, nc=nc))

    devices = jax.devices()[:n_cores]
    mesh = Mesh(_np.asarray(devices), ("core",))
    n_io = len(in_names) + len(out_names)
    fn = jax.jit(shard_map(_body, mesh=mesh,
                           in_specs=(PartitionSpec("core"),) * n_io,
                           out_specs=(PartitionSpec("core"),) * len(out_names),
                           check_rep=False), keep_unused=True)

    def run(in_maps):
        concat_in = [_np.concatenate([_np.asarray(in_maps[c][n])
                                      for c in range(n_cores)], axis=0)
                     for n in in_names]
        concat_zero = [_np.zeros((n_cores * z.shape[0], *z.shape[1:]), z.dtype)
                       for z in zero_outs]
        outs = fn(*concat_in, *concat_zero)
        return [{name: _np.asarray(outs[i]).reshape(
                    n_cores, *out_avals[i].shape)[c]
                 for i, name in enumerate(out_names)}
                for c in range(n_cores)]

    return run


def _patch_far(d2, xin):
    """Re-solve every voxel with banded d^2 >= (R+1)^2 exactly via a
    radius-6 window search (any voxel the band could have gotten wrong is in
    this set: the banded value only over-estimates, and a band violation
    implies true distance >= R+1).  Returns (patched d2, ok); ok=False means
    some such voxel has no zero within distance < 6 (or there are
    implausibly many) and the caller must use the full exact fallback."""
    sus = np.argwhere(d2 >= (R + 1) ** 2 - 0.5)
    if sus.shape[0] == 0:
        return d2, True
    if sus.shape[0] > 1_000_000:
        return d2, False
    rr = 6
    zp = np.pad(xin == 0, rr, constant_values=False)
    og = np.arange(-rr, rr + 1, dtype=np.int32)
    ob, oh, ow = np.meshgrid(og, og, og, indexing="ij")
    w2 = (ob * ob + oh * oh + ow * ow).astype(np.float32).ravel()
    obf = (ob.ravel() + rr)[None, :]
    ohf = (oh.ravel() + rr)[None, :]
    owf = (ow.ravel() + rr)[None, :]
    vals = np.empty(sus.shape[0], np.float32)
    CH = 2048
    for i0 in range(0, sus.shape[0], CH):
        s = sus[i0:i0 + CH].astype(np.int32)
        win = zp[s[:, 0:1] + obf, s[:, 1:2] + ohf, s[:, 2:3] + owf]
        d2w = np.where(win, w2[None, :], np.float32(1e9)).min(axis=1)
        if (d2w > 35.5).any():
            return d2, False
        vals[i0:i0 + CH] = d2w
    d2[sus[:, 0], sus[:, 1], sus[:, 2]] = vals
    return d2, True


def kernel(x):
    global _BUILT, _RUNNER
    x = np.asarray(x)
    assert x.shape == (B, H, W)
    if x.dtype != np.float32:
        x = x.astype(np.float32)

    if _BUILT is None:
        _BUILT = _build()
    (nc5,) = _BUILT
    if _RUNNER is None:
        _RUNNER = _make_runner(nc5, NCORES)
    LAST_RESULTS.clear()

    nan_mask = np.isnan(x)
    xin = np.where(nan_mask, np.float32(1.0), x) if nan_mask.any() else x
    if np.signbit(xin).any():
        xin = np.abs(xin)   # device binarize is Relu(32x): needs x >= 0
    # f16 on the wire: binary values survive the cast exactly, and the device
    # binarize must not see NaNs (they would poison the W scans).
    xp = np.pad(xin.astype(np.float16), ((0, 0), (HB, HB), (0, 0)),
                constant_values=1.0)
    in5 = [{"xs5": np.ascontiguousarray(xp[:, k * HS:k * HS + HE, :])}
           for k in range(NCORES)]
    results = _RUNNER(in5)
    outt = np.concatenate([results[k]["ot5"] for k in range(NCORES)], axis=2)

    d2 = outt.transpose(1, 2, 0).astype(np.float32)   # (w,b,h) -> (b,h,w)
    d2, ok = _patch_far(d2, xin)
    out = np.sqrt(d2) if ok else _host_exact_edt(xin)

    if nan_mask.any():
        out = np.where(nan_mask, np.float32(np.nan), out)
    return out


# revision 21
# speedup vs baseline: 2.0298x; 1.0744x over previous
"""Trainium2 Bass kernel: exact 3D Euclidean distance transform of a binary
(16, 512, 512) float32 volume — distance from every nonzero voxel to the
nearest zero voxel over ALL three axes (batch participates in the metric),
matching scipy.ndimage.distance_transform_edt on the full array.

Fast path / slow path split:
  Device (this kernel): separable EDT with an exact W pass (fwd/bwd
  saturating scans) and parabola min-plus passes along H and B banded at
  radius R=2.  This is exact for every voxel whose true distance is < R+1
  (its optimal per-axis offsets are <= floor(d) <= R), i.e. for ~99% of
  voxels at the 5%-background density this module targets.
  Host: every voxel with device d^2 >= (R+1)^2 (any voxel the band could
  have gotten wrong necessarily lands in this set, because the banded value
  only ever over-estimates and a band violation implies true d >= R+1) is
  re-solved exactly by a vectorized radius-6 window search; if any such
  voxel has no zero within distance < 6 the whole volume falls back to an
  exact host EDT.  The patched result is exact everywhere, for any input.

Device pipeline (values are small integers <= CLAMP^2+8, exact in fp16,
which unlocks the DVE 2x/4x perf modes):
  pass W: 1D nearest-zero distance along W via fwd/bwd scans
          (tensor_tensor_scan, DVE-only op), squared during the PSUM
          evacuation of a PE transpose (ACT Square).
  pass H: banded parabola min-plus along H (radius 2).
  pass B: banded parabola min-plus along B (radius 2).
  Output is d^2 in fp16, w-major; the host does the final sqrt.

Engine split (only DVE and ACT can do general elementwise work on TRN2
silicon; Pool rejects TensorTensor/TensorScalarPtr at codegen):
  DVE: scans (1x), every min (tensor_tensor, 2x), first binarize chunk and
       the B-pass +1 adds (tensor_scalar, 4x).
  ACT: binarize Relu(CLAMP*x), PSUM evacuation fused with Square, +s^2 adds
       (Copy + bias).
  PE:  transposes.  SP(sync): input DMA issue.  Pool: constants only.
The H pass is split into (b-chunk x j-half) pieces whose b-chunks only
depend on already-evacuated thirds of the squared field; pairs are emitted
before folds so the serial fold chain never starves; the B pass runs per
j-half / per j so each output DMA starts as soon as its slice is final.

Sharding: data-parallel over H (8 slabs of 64 rows); the W-scan needs full
W and the B-pass full B, which each slab has; the H-pass needs a 4-row
input halo (host pads with foreground).  No cross-core communication.
I/O is fp16: binary input survives the cast exactly; d^2 outputs are small
exact integers.

Hardware quirk: several instruction encodings accept only ONE semaphore
wait; _split_multi_waits hoists extra waits onto same-engine NoOp carriers.
"""
import numpy as np

B, H, W = 16, 512, 512
NCORES = 8
HS = H // NCORES          # 64 interior rows per core
P = 128
CLAMP = 32.0
R = 2                     # band radius of the H and B passes

HB = 4                    # input halo rows per side (>= R, tile-aligning)
HE = HS + 2 * HB          # 72 extended rows per core
N_T = (B * HE) // P       # 9 scan tiles
N_J = W // P              # 4 w-groups
CE = B * HE               # 1152 transposed lines per j-group
C = B * HS                # 1024 interior (b,h) elements per j-group
NG = 3                    # PSUM evacuation groups per j (3 tiles each)

_BUILT = None
LAST_RESULTS = []   # kept for the test harness's profiling hook


def _k5_body(tc, out_d, xs_d):
    """Fused single-launch banded-EDT device pass.

    xs_d:  [16, HE, 512] f16 dram (ExternalInput, host-padded h-slab)
    out_d: [512, 16, HS] f16 dram (ExternalOutput), squared distances,
           w-major
    """
    import concourse.mybir as mybir

    nc = tc.nc
    f16 = mybir.dt.float16
    Alu = mybir.AluOpType
    Act = mybir.ActivationFunctionType

    from concourse.masks import make_identity

    with tc.tile_pool(name="const", bufs=1) as cpool, \
         tc.tile_pool(name="big", bufs=1) as bpool, \
         tc.tile_pool(name="htmp", bufs=12) as hpool, \
         tc.tile_pool(name="btmp", bufs=6) as tbpool, \
         tc.tile_pool(name="psum", bufs=4, space="PSUM") as ppool, \
         tc.tile_pool(name="psumw", bufs=1, space="PSUM") as ppoolw:

        ones = cpool.tile([P, W], f16)
        nc.gpsimd.memset(ones[:], 1.0)
        ident = cpool.tile([P, P], f16)
        make_identity(nc, ident[:])
        # dummy transpose so PE observes the gpsimd-built identity before the
        # real transposes (keeps every matmul at <= 1 semaphore wait)
        psw = ppoolw.tile([P, P], f16)
        nc.tensor.transpose(psw[:], ident[:], ident[:])

        XH = bpool.tile([P, N_T * W], f16)      # raw x, f16
        AALL = bpool.tile([P, N_T * W], f16)    # d0 = (x != 0) * CLAMP
        FALL = bpool.tile([P, N_T * W], f16)    # fwd scan
        DALL = bpool.tile([P, N_T * W], f16)    # bwd scan of fwd = 1D dist

        # input DMAs on the sync queue (its trigger issue starts immediately;
        # the Pool queue is busy building the identity).  First chunk is one
        # tile so the first binarize + scan start as early as possible.
        xflat = xs_d.rearrange("b h w -> (b h) w")
        chunks = [(0, 1), (1, 2), (3, 2), (5, 2), (7, 2)]
        for t0, k in chunks:
            if k == 1:
                nc.sync.dma_start(XH[:, W * t0: W * (t0 + 1)],
                                  xflat[P * t0: P * (t0 + 1)])
            else:
                nc.sync.dma_start(
                    XH[:, W * t0: W * (t0 + k)].rearrange(
                        "p (g w) -> p g w", g=k),
                    xflat[P * t0: P * (t0 + k)].rearrange(
                        "(g pp) w -> pp g w", g=k))
        # binarize: first three tiles on DVE (4x tensor_scalar, shortest
        # latency to the early scans), the rest on ACT (Relu(CLAMP*x)).
        nc.vector.tensor_scalar(AALL[:, 0:W], XH[:, 0:W], 0.0, CLAMP,
                                Alu.not_equal, Alu.mult)
        nc.vector.tensor_scalar(AALL[:, W:3 * W], XH[:, W:3 * W], 0.0, CLAMP,
                                Alu.not_equal, Alu.mult)
        for t0, k in chunks[2:]:
            nc.scalar.activation(AALL[:, W * t0: W * (t0 + k)],
                                 XH[:, W * t0: W * (t0 + k)],
                                 Act.Relu, scale=CLAMP)

        for t in range(N_T):
            fa = FALL[:, W * t: W * (t + 1)]
            nc.vector.tensor_tensor_scan(
                fa, ones[:, 0:W], AALL[:, W * t: W * (t + 1)], CLAMP,
                Alu.add, Alu.min)
            nc.vector.tensor_tensor_scan(
                DALL[:, W * t: W * (t + 1)][:, ::-1], ones[:, 0:W],
                fa[:, ::-1], CLAMP, Alu.add, Alu.min)

        # transpose + evacuate-with-Square, in NG groups of 3 scan tiles per
        # j-group so the H pass can start on low-b chunks early.
        SQ = bpool.tile([P, N_J * CE], f16)     # w lines x (j, b, h72)
        GT = N_T // NG                          # 3 tiles per group
        for g in range(NG):
            for j in range(N_J):
                ps = ppool.tile([P, GT * P], f16, tag="ps")
                for tt in range(GT):
                    t = g * GT + tt
                    nc.tensor.transpose(
                        ps[:, P * tt: P * (tt + 1)],
                        DALL[:, W * t + P * j: W * t + P * (j + 1)],
                        ident[:])
                nc.scalar.activation(
                    SQ[:, CE * j + GT * P * g: CE * j + GT * P * (g + 1)],
                    ps[:], Act.Square)

        sq5 = SQ[:].rearrange("p (j b h) -> p j b h", j=N_J, b=B)
        ACH = bpool.tile([P, N_J * C], f16)
        ah4 = ACH[:].rearrange("p (j b h) -> p j b h", j=N_J, b=B)
        ACC = bpool.tile([P, N_J * C], f16)
        ac4 = ACC[:].rearrange("p (j b h) -> p j b h", j=N_J, b=B)

        # b-chunks: (0:5) lies inside evacuation group 0 (lines < 360 < 384),
        # (5:10) inside groups 0-1, (10:16) needs all three.
        BCH = [(0, 5), (5, 10), (10, 16)]

        def h_pairs(bc, jh):
            """Pass H pair mins (DVE) + in-place +s^2 (ACT) on one
            (b-chunk, j-half).  All six chunks' pairs are emitted before any
            fold so the DVE fold chains never starve."""
            b0, b1 = BCH[bc]
            nb = b1 - b0
            sq = sq5[:, 2 * jh:2 * (jh + 1), b0:b1, :]
            ts = []
            for s in range(1, R + 1):
                lo = sq[:, :, :, HB - s:HB - s + HS]
                hi = sq[:, :, :, HB + s:HB + s + HS]
                t_ = hpool.tile([P, 2 * nb * HS], f16, tag="hq")
                tv = t_[:].rearrange("p (j b h) -> p j b h", j=2, b=nb)
                nc.vector.tensor_tensor(tv, lo, hi, Alu.min)
                ts.append(tv)
            for s in range(1, R + 1):
                nc.scalar.activation(ts[s - 1], ts[s - 1], Act.Copy,
                                     bias=float(s * s))
            return ts

        def h_folds(bc, jh, ts):
            """Pass H fold chain (DVE) on one (b-chunk, j-half)."""
            b0, b1 = BCH[bc]
            sq = sq5[:, 2 * jh:2 * (jh + 1), b0:b1, :]
            a = ah4[:, 2 * jh:2 * (jh + 1), b0:b1, :]
            ctr = sq[:, :, :, HB:HB + HS]
            nc.vector.tensor_tensor(a, ts[0], ctr, Alu.min)
            for s in range(2, R + 1):
                nc.vector.tensor_tensor(a, ts[s - 1], a, Alu.min)

        def b_adds(j0, nj):
            """ACT-side prep for pass B on j-groups [j0, j0+nj): the b = B-1
            strip of the accumulator (its center term) and the shared +s^2
            tensors for s >= 2 (s = 1 is a DVE 4x tensor_scalar in
            b_folds)."""
            a = ah4[:, j0:j0 + nj]
            c = ac4[:, j0:j0 + nj]
            nc.scalar.activation(c[:, :, B - 1:B, :], a[:, :, B - 1:B, :],
                                 Act.Copy, bias=0.0)
            tbs = []
            for s in range(2, R + 1):
                tb = tbpool.tile([P, nj * C], f16, tag=f"tb{nj}")
                tv = tb[:].rearrange("p (j b h) -> p j b h", j=nj, b=B)
                nc.scalar.activation(tv, a, Act.Copy, bias=float(s * s))
                tbs.append(tv)
            return tbs

        def b_folds(j0, nj, tbs):
            """Pass B directional folds on j-groups [j0, j0+nj) (DVE).  The
            s=1 add runs on DVE (4x) so the chain starts without ACT."""
            a = ah4[:, j0:j0 + nj]
            c = ac4[:, j0:j0 + nj]
            tb1 = tbpool.tile([P, nj * C], f16, tag=f"tbd{nj}")
            t1 = tb1[:].rearrange("p (j b h) -> p j b h", j=nj, b=B)
            nc.vector.tensor_scalar(t1, a, 1.0, None, Alu.add)
            for s in range(1, R + 1):
                tv = t1 if s == 1 else tbs[s - 2]
                bc = B - s
                if s == 1:
                    nc.vector.tensor_tensor(c[:, :, 0:bc, :],
                                            tv[:, :, s:B, :],
                                            a[:, :, 0:bc, :], Alu.min)
                else:
                    nc.vector.tensor_tensor(c[:, :, 0:bc, :],
                                            tv[:, :, s:B, :],
                                            c[:, :, 0:bc, :], Alu.min)
                nc.vector.tensor_tensor(c[:, :, s:B, :], tv[:, :, 0:bc, :],
                                        c[:, :, s:B, :], Alu.min)

        outd = out_d.rearrange("(j p) b h -> p j (b h)", p=P)
        accs = ACC[:].rearrange("p (j c) -> p j c", j=N_J)

        # DVE order keeps the engine stall-free: every chunk's pairs first
        # (they only depend on evacuations), then the jh0 fold chains, the
        # jh0 B pass + its DMA, the jh1 fold chains, then per-j B chains
        # each followed by its own DMA so the tail drains incrementally.
        ts = {}
        for bc, jh in [(0, 0), (1, 0), (2, 0), (0, 1), (1, 1), (2, 1)]:
            ts[(bc, jh)] = h_pairs(bc, jh)
        for bc in range(3):
            h_folds(bc, 0, ts[(bc, 0)])
        tbs0 = b_adds(0, 2)
        b_folds(0, 2, tbs0)
        nc.scalar.dma_start(outd[:, 0:2], accs[:, 0:2])
        for bc in range(3):
            h_folds(bc, 1, ts[(bc, 1)])
        tbs2 = b_adds(2, 1)
        b_folds(2, 1, tbs2)
        nc.sync.dma_start(outd[:, 2:3], accs[:, 2:3])
        tbs3 = b_adds(3, 1)
        b_folds(3, 1, tbs3)
        nc.sync.dma_start(outd[:, 3:4], accs[:, 3:4])


def _split_multi_waits(nc):
    """Walrus in this toolchain encodes at most ONE sync wait per hardware
    instruction.  Hoist extra waits onto same-engine NoOp carriers inserted
    immediately before the over-subscribed instruction (program order on the
    engine preserves the semantics exactly)."""
    import concourse.mybir as mybir

    n = 0
    for fn in nc.m.functions:
        for blk in fn.blocks:
            insts = blk.instructions
            out = []
            for inst in insts:
                si = inst.sync_info
                if si is not None and len(si.on_wait) > 1:
                    waits = list(si.on_wait)
                    for w in waits[:-1]:
                        nop = mybir.InstNoOp(
                            name=f"waitsplit-{n}", ins=[], outs=[])
                        n += 1
                        nop.engine = inst.engine
                        nop.sync_info = mybir.SyncInfo(
                            on_wait=[w], on_update=[])
                        out.append(nop)
                    inst.sync_info = mybir.SyncInfo(
                        on_wait=[waits[-1]], on_update=list(si.on_update))
                out.append(inst)
            blk.instructions = out
    return n


def _make_tc_class():
    """TileContext whose kernel-tail drain is split into one drain per proc.

    The stock tail emits a single sync-engine Drain waiting on every
    outstanding processor; this walrus build only encodes ONE sync wait per
    instruction, so the aggregated drain fails codegen.  Semantics are
    identical — the waits just land on consecutive Drain instructions.
    """
    import concourse.tile as tile
    from concourse.vector_clock import ScopedClock, VectorClock

    class SplitDrainTileContext(tile.TileContext):
        def _drain_and_barrier(self, tick_clock, wait_clock):
            gvc = tick_clock.global_clock
            for proc in range(len(gvc)):
                t = gvc[proc]
                if t <= 0:
                    continue
                d = self.nc.sync.drain()
                sv = VectorClock([0] * len(gvc))
                sv.require_at_least(proc, t)
                wait_clock.add_sem_waits(d.ins, ScopedClock({None: sv}))
            self.nc.all_engine_barrier()
            assert self.sems is not None
            popped = self.nc._tile_sem_poison_stack.pop()
            assert popped is self._sem_poison
            self.nc.clear_and_free_semaphores(
                list(self.sems.allocated().values()))
            self.nc.all_engine_barrier()

    return SplitDrainTileContext


def _build():
    """Build the fused Bass module (done once per process)."""
    import concourse.bass as bass
    import concourse.mybir as mybir

    f16 = mybir.dt.float16
    TC = _make_tc_class()

    nc5 = bass.Bass("TRN2", debug=False, num_devices=NCORES)
    xs5_d = nc5.dram_tensor("xs5", [B, HE, W], f16,
                            kind="ExternalInput").ap()
    ot5_d = nc5.dram_tensor("ot5", [W, B, HS], f16,
                            kind="ExternalOutput").ap()
    with TC(nc5) as tc:
        _k5_body(tc, ot5_d, xs5_d)
    _split_multi_waits(nc5)
    return (nc5,)


def _host_exact_edt(x):
    """Exact host fallback: banded numpy EDT with growing radius (f32)."""
    INF = np.float32(1e9)
    r = 8
    while True:
        d0 = np.where(x != 0, INF, np.float32(0.0))
        fwd = np.empty_like(d0)
        st = np.full(d0.shape[:2], INF, np.float32)
        for w in range(W):
            st = np.minimum(st + 1.0, d0[:, :, w]); fwd[:, :, w] = st
        st = np.full(d0.shape[:2], INF, np.float32)
        bwd = np.empty_like(d0)
        for w in range(W - 1, -1, -1):
            st = np.minimum(st + 1.0, d0[:, :, w]); bwd[:, :, w] = st
        d2 = np.minimum(fwd, bwd) ** 2
        for axis in (0, 1):
            src = d2
            acc = src.copy()
            rr = min(r, x.shape[axis] - 1)
            for s in range(1, rr + 1):
                sl_lo = [slice(None)] * 3
                sl_hi = [slice(None)] * 3
                sl_lo[axis] = slice(0, x.shape[axis] - s)
                sl_hi[axis] = slice(s, None)
                np.minimum(acc[tuple(sl_lo)], src[tuple(sl_hi)] + s * s,
                           out=acc[tuple(sl_lo)])
                np.minimum(acc[tuple(sl_hi)], src[tuple(sl_lo)] + s * s,
                           out=acc[tuple(sl_hi)])
            d2 = acc
        out = np.sqrt(d2)
        # exact when every per-axis offset fits in the band; r >= max dim
        # means the bands are complete regardless of the value of out
        if out.max() <= r or r >= max(x.shape):
            return out.astype(np.float32)
        r *= 2


_RUNNER = None


def _make_runner(nc, n_cores):
    """Build the sharded PJRT callable once (run_bass_kernel_spmd re-traces
    and re-jits on every call; caching saves ~1 s per kernel() invocation)."""
    import jax
    import numpy as _np
    from jax.sharding import Mesh, PartitionSpec
    from jax.experimental.shard_map import shard_map
    import concourse.mybir as mybir
    from concourse import bass2jax

    bass2jax.install_neuronx_cc_hook()
    partition_name = (nc.partition_id_tensor.name
                      if nc.partition_id_tensor else None)
    in_names, out_names, out_avals, zero_outs = [], [], [], []
    for alloc in nc.m.functions[0].allocations:
        if not isinstance(alloc, mybir.MemoryLocationSet):
            continue
        name = alloc.memorylocations[0].name
        if alloc.kind == "ExternalInput":
            if name != partition_name:
                in_names.append(name)
        elif alloc.kind == "ExternalOutput":
            out_avals.append(jax.core.ShapedArray(
                tuple(alloc.tensor_shape), mybir.dt.np(alloc.dtype)))
            out_names.append(name)
            zero_outs.append(_np.zeros(tuple(alloc.tensor_shape),
                                       mybir.dt.np(alloc.dtype)))
    all_in = list(in_names) + list(out_names)
    if partition_name is not None:
        all_in.append(partition_name)

    def _body(*args):
        operands = list(args)
        if partition_name is not None:
            operands.append(bass2jax.partition_id_tensor())
        return tuple(bass2jax._bass_exec_p.bind(
            *operands, out_avals=tuple(out_avals), in_names=tuple(all_in),
            out_names=tuple(out_names), lowering_input_output_aliases=(),
            sim_require_finite=True, sim_require_nnan=True, nc=nc))

    devices = jax.devices()[:n_cores]
    mesh = Mesh(_np.asarray(devices), ("core",))
    n_io = len(in_names) + len(out_names)
    fn = jax.jit(shard_map(_body, mesh=mesh,
                           in_specs=(PartitionSpec("core"),) * n_io,
                           out_specs=(PartitionSpec("core"),) * len(out_names),
                           check_rep=False), keep_unused=True)

    def run(in_maps):
        concat_in = [_np.concatenate([_np.asarray(in_maps[c][n])
                                      for c in range(n_cores)], axis=0)
                     for n in in_names]
        concat_zero = [_np.zeros((n_cores * z.shape[0], *z.shape[1:]), z.dtype)
                       for z in zero_outs]
        outs = fn(*concat_in, *concat_zero)
        return [{name: _np.asarray(outs[i]).reshape(
                    n_cores, *out_avals[i].shape)[c]
                 for i, name in enumerate(out_names)}
                for c in range(n_cores)]

    return run


def _patch_far(d2, xin):
    """Re-solve every voxel with banded d^2 >= (R+1)^2 exactly via a
    radius-6 window search (any voxel the band could have gotten wrong is in
    this set: the banded value only over-estimates, and a band violation
    implies true distance >= R+1).  Returns (patched d2, ok); ok=False means
    some such voxel has no zero within distance < 6 (or there are
    implausibly many) and the caller must use the full exact fallback."""
    sus = np.argwhere(d2 >= (R + 1) ** 2 - 0.5)
    if sus.shape[0] == 0:
        return d2, True
    if sus.shape[0] > 1_000_000:
        return d2, False
    rr = 6
    zp = np.pad(xin == 0, rr, constant_values=False)
    og = np.arange(-rr, rr + 1, dtype=np.int32)
    ob, oh, ow = np.meshgrid(og, og, og, indexing="ij")
    w2 = (ob * ob + oh * oh + ow * ow).astype(np.float32).ravel()
    obf = (ob.ravel() + rr)[None, :]
    ohf = (oh.ravel() + rr)[None, :]
    owf = (ow.ravel() + rr)[None, :]
    vals = np.empty(sus.shape[0], np.float32)
    CH = 2048
    for i0 in range(0, sus.shape[0], CH):
        s = sus[i0:i0 + CH].astype(np.int32)
        win = zp[s[:, 0:1] + obf, s[:, 1:2] + ohf, s[:, 2:3] + owf]
        d2w = np.where(win, w2[None, :], np.float32(1e9)).min(axis=1)
        if (d2w > 35.5).any():
            return d2, False
        vals[i0:i0 + CH] = d2w
    d2[sus[:, 0], sus[:, 1], sus[:, 2]] = vals
    return d2, True


def kernel(x):
    global _BUILT, _RUNNER
    x = np.asarray(x)
    assert x.shape == (B, H, W)
    if x.dtype != np.float32:
        x = x.astype(np.float32)

    if _BUILT is None:
        _BUILT = _build()
    (nc5,) = _BUILT
    if _RUNNER is None:
        _RUNNER = _make_runner(nc5, NCORES)
    LAST_RESULTS.clear()

    nan_mask = np.isnan(x)
    xin = np.where(nan_mask, np.float32(1.0), x) if nan_mask.any() else x
    if np.signbit(xin).any():
        xin = np.abs(xin)   # device binarize is Relu(32x): needs x >= 0
    # f16 on the wire: binary values survive the cast exactly, and the device
    # binarize must not see NaNs (they would poison the W scans).
    xp = np.pad(xin.astype(np.float16), ((0, 0), (HB, HB), (0, 0)),
                constant_values=1.0)
    in5 = [{"xs5": np.ascontiguousarray(xp[:, k * HS:k * HS + HE, :])}
           for k in range(NCORES)]
    results = _RUNNER(in5)
    outt = np.concatenate([results[k]["ot5"] for k in range(NCORES)], axis=2)

    d2 = outt.transpose(1, 2, 0).astype(np.float32)   # (w,b,h) -> (b,h,w)
    d2, ok = _patch_far(d2, xin)
    out = np.sqrt(d2) if ok else _host_exact_edt(xin)

    if nan_mask.any():
        out = np.where(nan_mask, np.float32(np.nan), out)
    return out


# revision 29
# speedup vs baseline: 2.0818x; 1.0256x over previous
"""Trainium2 Bass kernel: exact 3D Euclidean distance transform of a binary
(16, 512, 512) float32 volume — distance from every nonzero voxel to the
nearest zero voxel over ALL three axes (batch participates in the metric),
matching scipy.ndimage.distance_transform_edt on the full array.

Fast path / slow path split:
  Device (this kernel): separable EDT with an exact W pass (fwd/bwd
  saturating scans) and parabola min-plus passes along H and B banded at
  radius R=2.  This is exact for every voxel whose true distance is < R+1
  (its optimal per-axis offsets are <= floor(d) <= R), i.e. for ~99% of
  voxels at the 5%-background density this module targets.
  Host: every voxel with device d^2 >= (R+1)^2 (any voxel the band could
  have gotten wrong necessarily lands in this set, because the banded value
  only ever over-estimates and a band violation implies true d >= R+1) is
  re-solved exactly by a vectorized radius-6 window search; if any such
  voxel has no zero within distance < 6 the whole volume falls back to an
  exact host EDT.  The patched result is exact everywhere, for any input.

Device pipeline (values are small integers <= CLAMP^2+8, exact in fp16,
which unlocks the DVE 2x/4x perf modes):
  pass W: 1D nearest-zero distance along W via fwd/bwd scans
          (tensor_tensor_scan, DVE-only op), squared during the PSUM
          evacuation of a PE transpose (ACT Square).
  pass H: banded parabola min-plus along H (radius 2).
  pass B: banded parabola min-plus along B (radius 2).
  Output is d^2 in fp16, w-major; the host does the final sqrt.

Engine split (only DVE and ACT can do general elementwise work on TRN2
silicon; Pool rejects TensorTensor/TensorScalarPtr at codegen):
  DVE: scans (1x), every min (tensor_tensor, 2x), first binarize chunk and
       the B-pass +1 adds (tensor_scalar, 4x).
  ACT: binarize Relu(CLAMP*x), PSUM evacuation fused with Square, +s^2 adds
       (Copy + bias).
  PE:  transposes.  SP(sync): input DMA issue.  Pool: constants only.
The H pass is split into (b-chunk x j-half) pieces whose b-chunks only
depend on already-evacuated thirds of the squared field; pairs are emitted
before folds so the serial fold chain never starves; the B pass runs per
j-half / per j so each output DMA starts as soon as its slice is final.

Sharding: data-parallel over H (8 slabs of 64 rows); the W-scan needs full
W and the B-pass full B, which each slab has; the H-pass needs a 4-row
input halo (host pads with foreground).  No cross-core communication.
I/O is fp16: binary input survives the cast exactly; d^2 outputs are small
exact integers.

Hardware quirk: several instruction encodings accept only ONE semaphore
wait; _split_multi_waits hoists extra waits onto same-engine NoOp carriers.
"""
import numpy as np

B, H, W = 16, 512, 512
NCORES = 8
HS = H // NCORES          # 64 interior rows per core
P = 128
CLAMP = 32.0
R = 2                     # band radius of the H and B passes

HB = 4                    # input halo rows per side (>= R, tile-aligning)
HE = HS + 2 * HB          # 72 extended rows per core
N_T = (B * HE) // P       # 9 scan tiles
N_J = W // P              # 4 w-groups
CE = B * HE               # 1152 transposed lines per j-group
C = B * HS                # 1024 interior (b,h) elements per j-group
NG = 3                    # PSUM evacuation groups per j (3 tiles each)

_BUILT = None
LAST_RESULTS = []   # kept for the test harness's profiling hook


def _k5_body(tc, out_d, xs_d):
    """Fused single-launch banded-EDT device pass.

    xs_d:  [16, HE, 512] f16 dram (ExternalInput, host-padded h-slab)
    out_d: [512, 16, HS] f16 dram (ExternalOutput), squared distances,
           w-major
    """
    import concourse.mybir as mybir

    nc = tc.nc
    f16 = mybir.dt.float16
    Alu = mybir.AluOpType
    Act = mybir.ActivationFunctionType

    from concourse.masks import make_identity

    with tc.tile_pool(name="const", bufs=1) as cpool, \
         tc.tile_pool(name="big", bufs=1) as bpool, \
         tc.tile_pool(name="htmp", bufs=12) as hpool, \
         tc.tile_pool(name="btmp", bufs=6) as tbpool, \
         tc.tile_pool(name="psum", bufs=4, space="PSUM") as ppool, \
         tc.tile_pool(name="psumw", bufs=1, space="PSUM") as ppoolw:

        ones = cpool.tile([P, W], f16)
        nc.gpsimd.memset(ones[:], 1.0)
        ident = cpool.tile([P, P], f16)
        make_identity(nc, ident[:])
        # dummy transpose so PE observes the gpsimd-built identity before the
        # real transposes (keeps every matmul at <= 1 semaphore wait)
        psw = ppoolw.tile([P, P], f16)
        nc.tensor.transpose(psw[:], ident[:], ident[:])

        AALL = bpool.tile([P, N_T * W], f16)    # d0 = (x != 0) * CLAMP (host)
        FALL = bpool.tile([P, N_T * W], f16)    # fwd scan
        DALL = bpool.tile([P, N_T * W], f16)    # bwd scan of fwd = 1D dist

        # input DMAs on the sync queue (its trigger issue starts immediately;
        # the Pool queue is busy building the identity).  The host sends the
        # already-binarized (x != 0) * CLAMP field, so the first scan starts
        # as soon as the first (single-tile) chunk lands.
        xflat = xs_d.rearrange("b h w -> (b h) w")
        chunks = [(0, 1), (1, 2), (3, 2), (5, 2), (7, 2)]
        for t0, k in chunks:
            if k == 1:
                nc.sync.dma_start(AALL[:, W * t0: W * (t0 + 1)],
                                  xflat[P * t0: P * (t0 + 1)])
            else:
                nc.sync.dma_start(
                    AALL[:, W * t0: W * (t0 + k)].rearrange(
                        "p (g w) -> p g w", g=k),
                    xflat[P * t0: P * (t0 + k)].rearrange(
                        "(g pp) w -> pp g w", g=k))

        for t in range(N_T):
            fa = FALL[:, W * t: W * (t + 1)]
            nc.vector.tensor_tensor_scan(
                fa, ones[:, 0:W], AALL[:, W * t: W * (t + 1)], CLAMP,
                Alu.add, Alu.min)
            nc.vector.tensor_tensor_scan(
                DALL[:, W * t: W * (t + 1)][:, ::-1], ones[:, 0:W],
                fa[:, ::-1], CLAMP, Alu.add, Alu.min)

        # transpose + evacuate-with-Square, in NG groups of 3 scan tiles per
        # j-group so the H pass can start on low-b chunks early.
        SQ = bpool.tile([P, N_J * CE], f16)     # w lines x (j, b, h72)
        GT = N_T // NG                          # 3 tiles per group
        for g in range(NG):
            for j in range(N_J):
                ps = ppool.tile([P, GT * P], f16, tag="ps")
                for tt in range(GT):
                    t = g * GT + tt
                    nc.tensor.transpose(
                        ps[:, P * tt: P * (tt + 1)],
                        DALL[:, W * t + P * j: W * t + P * (j + 1)],
                        ident[:])
                nc.scalar.activation(
                    SQ[:, CE * j + GT * P * g: CE * j + GT * P * (g + 1)],
                    ps[:], Act.Square)

        sq5 = SQ[:].rearrange("p (j b h) -> p j b h", j=N_J, b=B)
        ACH = bpool.tile([P, N_J * C], f16)
        ah4 = ACH[:].rearrange("p (j b h) -> p j b h", j=N_J, b=B)
        ACC = bpool.tile([P, N_J * C], f16)
        ac4 = ACC[:].rearrange("p (j b h) -> p j b h", j=N_J, b=B)

        # b-halves: (0:8) lies inside evacuation groups 0-1 (lines < 576 <
        # 768), (8:16) additionally needs group 2.
        BCH = [(0, 8), (8, 16)]

        def h_pairs(bc, jh):
            """Pass H pair mins (DVE) + in-place +s^2 (ACT) on one
            (b-half, j-half).  All pairs are emitted before any fold so the
            DVE fold chains never starve."""
            b0, b1 = BCH[bc]
            nb = b1 - b0
            sq = sq5[:, 2 * jh:2 * (jh + 1), b0:b1, :]
            ts = []
            for s in range(1, R + 1):
                lo = sq[:, :, :, HB - s:HB - s + HS]
                hi = sq[:, :, :, HB + s:HB + s + HS]
                t_ = hpool.tile([P, 2 * nb * HS], f16, tag="hq")
                tv = t_[:].rearrange("p (j b h) -> p j b h", j=2, b=nb)
                nc.vector.tensor_tensor(tv, lo, hi, Alu.min)
                ts.append(tv)
            for s in range(1, R + 1):
                nc.scalar.activation(ts[s - 1], ts[s - 1], Act.Copy,
                                     bias=float(s * s))
            return ts

        def h_folds(bc, jh, ts):
            """Pass H fold chain (DVE) on one (b-half, j-half)."""
            b0, b1 = BCH[bc]
            sq = sq5[:, 2 * jh:2 * (jh + 1), b0:b1, :]
            a = ah4[:, 2 * jh:2 * (jh + 1), b0:b1, :]
            ctr = sq[:, :, :, HB:HB + HS]
            nc.vector.tensor_tensor(a, ts[0], ctr, Alu.min)
            for s in range(2, R + 1):
                nc.vector.tensor_tensor(a, ts[s - 1], a, Alu.min)

        def b_adds(j0, nj):
            """ACT-side prep for pass B on j-groups [j0, j0+nj): the b = B-1
            strip of the accumulator (its center term) and the shared +s^2
            tensors for s >= 2 (s = 1 is a DVE 4x tensor_scalar in
            b_folds)."""
            a = ah4[:, j0:j0 + nj]
            c = ac4[:, j0:j0 + nj]
            nc.scalar.activation(c[:, :, B - 1:B, :], a[:, :, B - 1:B, :],
                                 Act.Copy, bias=0.0)
            tbs = []
            for s in range(2, R + 1):
                tb = tbpool.tile([P, nj * C], f16, tag=f"tb{nj}")
                tv = tb[:].rearrange("p (j b h) -> p j b h", j=nj, b=B)
                nc.scalar.activation(tv, a, Act.Copy, bias=float(s * s))
                tbs.append(tv)
            return tbs

        def b_folds(j0, nj, tbs):
            """Pass B directional folds on j-groups [j0, j0+nj) (DVE).  The
            s=1 add runs on DVE (4x) so the chain starts without ACT."""
            a = ah4[:, j0:j0 + nj]
            c = ac4[:, j0:j0 + nj]
            tb1 = tbpool.tile([P, nj * C], f16, tag=f"tbd{nj}")
            t1 = tb1[:].rearrange("p (j b h) -> p j b h", j=nj, b=B)
            nc.vector.tensor_scalar(t1, a, 1.0, None, Alu.add)
            for s in range(1, R + 1):
                tv = t1 if s == 1 else tbs[s - 2]
                bc = B - s
                if s == 1:
                    nc.vector.tensor_tensor(c[:, :, 0:bc, :],
                                            tv[:, :, s:B, :],
                                            a[:, :, 0:bc, :], Alu.min)
                else:
                    nc.vector.tensor_tensor(c[:, :, 0:bc, :],
                                            tv[:, :, s:B, :],
                                            c[:, :, 0:bc, :], Alu.min)
                nc.vector.tensor_tensor(c[:, :, s:B, :], tv[:, :, 0:bc, :],
                                        c[:, :, s:B, :], Alu.min)

        outd = out_d.rearrange("(j p) b h -> p j (b h)", p=P)
        accs = ACC[:].rearrange("p (j c) -> p j c", j=N_J)

        # DVE order keeps the engine stall-free: every chunk's pairs first
        # (they only depend on evacuations), then the jh0 fold chains, the
        # jh0 B pass + its DMA, the jh1 fold chains, then per-j B chains
        # each followed by its own DMA so the tail drains incrementally.
        ts = {}
        for bc, jh in [(0, 0), (1, 0), (0, 1), (1, 1)]:
            ts[(bc, jh)] = h_pairs(bc, jh)
        h_folds(0, 0, ts[(0, 0)])
        h_folds(1, 0, ts[(1, 0)])
        tbs0 = b_adds(0, 2)
        b_folds(0, 2, tbs0)
        nc.scalar.dma_start(outd[:, 0:2], accs[:, 0:2])
        h_folds(0, 1, ts[(0, 1)])
        h_folds(1, 1, ts[(1, 1)])
        tbs2 = b_adds(2, 1)
        b_folds(2, 1, tbs2)
        nc.sync.dma_start(outd[:, 2:3], accs[:, 2:3])
        tbs3 = b_adds(3, 1)
        b_folds(3, 1, tbs3)
        nc.sync.dma_start(outd[:, 3:4], accs[:, 3:4])


def _split_multi_waits(nc):
    """Walrus in this toolchain encodes at most ONE sync wait per hardware
    instruction.  Hoist extra waits onto same-engine NoOp carriers inserted
    immediately before the over-subscribed instruction (program order on the
    engine preserves the semantics exactly)."""
    import concourse.mybir as mybir

    n = 0
    for fn in nc.m.functions:
        for blk in fn.blocks:
            insts = blk.instructions
            out = []
            for inst in insts:
                si = inst.sync_info
                if si is not None and len(si.on_wait) > 1:
                    waits = list(si.on_wait)
                    for w in waits[:-1]:
                        nop = mybir.InstNoOp(
                            name=f"waitsplit-{n}", ins=[], outs=[])
                        n += 1
                        nop.engine = inst.engine
                        nop.sync_info = mybir.SyncInfo(
                            on_wait=[w], on_update=[])
                        out.append(nop)
                    inst.sync_info = mybir.SyncInfo(
                        on_wait=[waits[-1]], on_update=list(si.on_update))
                out.append(inst)
            blk.instructions = out
    return n


def _make_tc_class():
    """TileContext whose kernel-tail drain is split into one drain per proc.

    The stock tail emits a single sync-engine Drain waiting on every
    outstanding processor; this walrus build only encodes ONE sync wait per
    instruction, so the aggregated drain fails codegen.  Semantics are
    identical — the waits just land on consecutive Drain instructions.
    """
    import concourse.tile as tile
    from concourse.vector_clock import ScopedClock, VectorClock

    class SplitDrainTileContext(tile.TileContext):
        def _drain_and_barrier(self, tick_clock, wait_clock):
            gvc = tick_clock.global_clock
            for proc in range(len(gvc)):
                t = gvc[proc]
                if t <= 0:
                    continue
                d = self.nc.sync.drain()
                sv = VectorClock([0] * len(gvc))
                sv.require_at_least(proc, t)
                wait_clock.add_sem_waits(d.ins, ScopedClock({None: sv}))
            self.nc.all_engine_barrier()
            assert self.sems is not None
            popped = self.nc._tile_sem_poison_stack.pop()
            assert popped is self._sem_poison
            self.nc.clear_and_free_semaphores(
                list(self.sems.allocated().values()))
            self.nc.all_engine_barrier()

    return SplitDrainTileContext


def _build():
    """Build the fused Bass module (done once per process)."""
    import concourse.bass as bass
    import concourse.mybir as mybir

    f16 = mybir.dt.float16
    TC = _make_tc_class()

    nc5 = bass.Bass("TRN2", debug=False, num_devices=NCORES)
    xs5_d = nc5.dram_tensor("xs5", [B, HE, W], f16,
                            kind="ExternalInput").ap()
    ot5_d = nc5.dram_tensor("ot5", [W, B, HS], f16,
                            kind="ExternalOutput").ap()
    with TC(nc5) as tc:
        _k5_body(tc, ot5_d, xs5_d)
    _split_multi_waits(nc5)
    return (nc5,)


def _host_exact_edt(x):
    """Exact host fallback: banded numpy EDT with growing radius (f32)."""
    INF = np.float32(1e9)
    r = 8
    while True:
        d0 = np.where(x != 0, INF, np.float32(0.0))
        fwd = np.empty_like(d0)
        st = np.full(d0.shape[:2], INF, np.float32)
        for w in range(W):
            st = np.minimum(st + 1.0, d0[:, :, w]); fwd[:, :, w] = st
        st = np.full(d0.shape[:2], INF, np.float32)
        bwd = np.empty_like(d0)
        for w in range(W - 1, -1, -1):
            st = np.minimum(st + 1.0, d0[:, :, w]); bwd[:, :, w] = st
        d2 = np.minimum(fwd, bwd) ** 2
        for axis in (0, 1):
            src = d2
            acc = src.copy()
            rr = min(r, x.shape[axis] - 1)
            for s in range(1, rr + 1):
                sl_lo = [slice(None)] * 3
                sl_hi = [slice(None)] * 3
                sl_lo[axis] = slice(0, x.shape[axis] - s)
                sl_hi[axis] = slice(s, None)
                np.minimum(acc[tuple(sl_lo)], src[tuple(sl_hi)] + s * s,
                           out=acc[tuple(sl_lo)])
                np.minimum(acc[tuple(sl_hi)], src[tuple(sl_lo)] + s * s,
                           out=acc[tuple(sl_hi)])
            d2 = acc
        out = np.sqrt(d2)
        # exact when every per-axis offset fits in the band; r >= max dim
        # means the bands are complete regardless of the value of out
        if out.max() <= r or r >= max(x.shape):
            return out.astype(np.float32)
        r *= 2


_RUNNER = None


def _make_runner(nc, n_cores):
    """Build the sharded PJRT callable once (run_bass_kernel_spmd re-traces
    and re-jits on every call; caching saves ~1 s per kernel() invocation)."""
    import jax
    import numpy as _np
    from jax.sharding import Mesh, PartitionSpec
    from jax.experimental.shard_map import shard_map
    import concourse.mybir as mybir
    from concourse import bass2jax

    bass2jax.install_neuronx_cc_hook()
    partition_name = (nc.partition_id_tensor.name
                      if nc.partition_id_tensor else None)
    in_names, out_names, out_avals, zero_outs = [], [], [], []
    for alloc in nc.m.functions[0].allocations:
        if not isinstance(alloc, mybir.MemoryLocationSet):
            continue
        name = alloc.memorylocations[0].name
        if alloc.kind == "ExternalInput":
            if name != partition_name:
                in_names.append(name)
        elif alloc.kind == "ExternalOutput":
            out_avals.append(jax.core.ShapedArray(
                tuple(alloc.tensor_shape), mybir.dt.np(alloc.dtype)))
            out_names.append(name)
            zero_outs.append(_np.zeros(tuple(alloc.tensor_shape),
                                       mybir.dt.np(alloc.dtype)))
    all_in = list(in_names) + list(out_names)
    if partition_name is not None:
        all_in.append(partition_name)

    def _body(*args):
        operands = list(args)
        if partition_name is not None:
            operands.append(bass2jax.partition_id_tensor())
        return tuple(bass2jax._bass_exec_p.bind(
            *operands, out_avals=tuple(out_avals), in_names=tuple(all_in),
            out_names=tuple(out_names), lowering_input_output_aliases=(),
            sim_require_finite=True, sim_require_nnan=True, nc=nc))

    devices = jax.devices()[:n_cores]
    mesh = Mesh(_np.asarray(devices), ("core",))
    n_io = len(in_names) + len(out_names)
    fn = jax.jit(shard_map(_body, mesh=mesh,
                           in_specs=(PartitionSpec("core"),) * n_io,
                           out_specs=(PartitionSpec("core"),) * len(out_names),
                           check_rep=False), keep_unused=True)

    def run(in_maps):
        concat_in = [_np.concatenate([_np.asarray(in_maps[c][n])
                                      for c in range(n_cores)], axis=0)
                     for n in in_names]
        concat_zero = [_np.zeros((n_cores * z.shape[0], *z.shape[1:]), z.dtype)
                       for z in zero_outs]
        outs = fn(*concat_in, *concat_zero)
        return [{name: _np.asarray(outs[i]).reshape(
                    n_cores, *out_avals[i].shape)[c]
                 for i, name in enumerate(out_names)}
                for c in range(n_cores)]

    return run


def _patch_far(d2, xin):
    """Re-solve every voxel with banded d^2 >= (R+1)^2 exactly via a
    radius-6 window search (any voxel the band could have gotten wrong is in
    this set: the banded value only over-estimates, and a band violation
    implies true distance >= R+1).  Returns (patched d2, ok); ok=False means
    some such voxel has no zero within distance < 6 (or there are
    implausibly many) and the caller must use the full exact fallback."""
    sus = np.argwhere(d2 >= (R + 1) ** 2 - 0.5)
    if sus.shape[0] == 0:
        return d2, True
    if sus.shape[0] > 1_000_000:
        return d2, False
    rr = 6
    zp = np.pad(xin == 0, rr, constant_values=False)
    og = np.arange(-rr, rr + 1, dtype=np.int32)
    ob, oh, ow = np.meshgrid(og, og, og, indexing="ij")
    w2 = (ob * ob + oh * oh + ow * ow).astype(np.float32).ravel()
    obf = (ob.ravel() + rr)[None, :]
    ohf = (oh.ravel() + rr)[None, :]
    owf = (ow.ravel() + rr)[None, :]
    vals = np.empty(sus.shape[0], np.float32)
    CH = 2048
    for i0 in range(0, sus.shape[0], CH):
        s = sus[i0:i0 + CH].astype(np.int32)
        win = zp[s[:, 0:1] + obf, s[:, 1:2] + ohf, s[:, 2:3] + owf]
        d2w = np.where(win, w2[None, :], np.float32(1e9)).min(axis=1)
        if (d2w > 35.5).any():
            return d2, False
        vals[i0:i0 + CH] = d2w
    d2[sus[:, 0], sus[:, 1], sus[:, 2]] = vals
    return d2, True


def kernel(x):
    global _BUILT, _RUNNER
    x = np.asarray(x)
    assert x.shape == (B, H, W)
    if x.dtype != np.float32:
        x = x.astype(np.float32)

    if _BUILT is None:
        _BUILT = _build()
    (nc5,) = _BUILT
    if _RUNNER is None:
        _RUNNER = _make_runner(nc5, NCORES)
    LAST_RESULTS.clear()

    nan_mask = np.isnan(x)
    # The host binarizes: the device receives (x != 0) * CLAMP directly
    # (NaN != 0 is True, so NaN voxels are foreground, as in the reference).
    xin = (x != 0).astype(np.float32)     # 0 at zeros, 1 at foreground/NaN
    xp = np.pad((xin * CLAMP).astype(np.float16),
                ((0, 0), (HB, HB), (0, 0)),
                constant_values=np.float16(CLAMP))
    in5 = [{"xs5": np.ascontiguousarray(xp[:, k * HS:k * HS + HE, :])}
           for k in range(NCORES)]
    results = _RUNNER(in5)
    outt = np.concatenate([results[k]["ot5"] for k in range(NCORES)], axis=2)

    d2 = outt.transpose(1, 2, 0).astype(np.float32)   # (w,b,h) -> (b,h,w)
    d2, ok = _patch_far(d2, xin)
    out = np.sqrt(d2) if ok else _host_exact_edt(xin)

    if nan_mask.any():
        out = np.where(nan_mask, np.float32(np.nan), out)
    return out


# revision 39
# speedup vs baseline: 2.1558x; 1.0356x over previous
"""Trainium2 Bass kernel: exact 3D Euclidean distance transform of a binary
(16, 512, 512) float32 volume — distance from every nonzero voxel to the
nearest zero voxel over ALL three axes (batch participates in the metric),
matching scipy.ndimage.distance_transform_edt on the full array.

Fast path / slow path split:
  Device (this kernel): separable EDT with an exact W pass (fwd/bwd
  saturating scans) and parabola min-plus passes along H and B banded at
  radius R=2.  This is exact for every voxel whose true distance is < R+1
  (its optimal per-axis offsets are <= floor(d) <= R), i.e. for ~99% of
  voxels at the 5%-background density this module targets.
  Host: every voxel with device d^2 >= (R+1)^2 (any voxel the band could
  have gotten wrong necessarily lands in this set, because the banded value
  only ever over-estimates and a band violation implies true d >= R+1) is
  re-solved exactly by a vectorized radius-6 window search; if any such
  voxel has no zero within distance < 6 the whole volume falls back to an
  exact host EDT.  The patched result is exact everywhere, for any input.

Device pipeline (values are small integers <= CLAMP^2+8, exact in fp16,
which unlocks the DVE 2x/4x perf modes):
  pass W: 1D nearest-zero distance along W via fwd/bwd scans
          (tensor_tensor_scan, DVE-only op), squared during the PSUM
          evacuation of a PE transpose (ACT Square).
  pass H: banded parabola min-plus along H (radius 2).
  pass B: banded parabola min-plus along B (radius 2).
  Output is d^2 in fp16, w-major; the host does the final sqrt.

Engine split (only DVE and ACT can do general elementwise work on TRN2
silicon; Pool rejects TensorTensor/TensorScalarPtr at codegen):
  DVE: scans (1x), every min (tensor_tensor, 2x), first binarize chunk and
       the B-pass +1 adds (tensor_scalar, 4x).
  ACT: binarize Relu(CLAMP*x), PSUM evacuation fused with Square, +s^2 adds
       (Copy + bias).
  PE:  transposes.  SP(sync): input DMA issue.  Pool: constants only.
The H pass is split into (b-chunk x j-half) pieces whose b-chunks only
depend on already-evacuated thirds of the squared field; pairs are emitted
before folds so the serial fold chain never starves; the B pass runs per
j-half / per j so each output DMA starts as soon as its slice is final.

Sharding: data-parallel over H (8 slabs of 64 rows); the W-scan needs full
W and the B-pass full B, which each slab has; the H-pass needs a 4-row
input halo (host pads with foreground).  No cross-core communication.
I/O is fp16: binary input survives the cast exactly; d^2 outputs are small
exact integers.

Hardware quirk: several instruction encodings accept only ONE semaphore
wait; _split_multi_waits hoists extra waits onto same-engine NoOp carriers.
"""
import numpy as np

B, H, W = 16, 512, 512
NCORES = 8
HS = H // NCORES          # 64 interior rows per core
P = 128
CLAMP = 32.0
R = 2                     # band radius of the H and B passes

HB = 0                    # no input halo: the host re-solves the 4 rows
                          # around each slab boundary (and the volume edges)
HE = HS + 2 * HB          # 64 rows per core
N_T = (B * HE) // P       # 8 scan tiles
N_J = W // P              # 4 w-groups
CE = B * HE               # 1024 transposed lines per j-group
C = B * HS                # 1024 interior (b,h) elements per j-group
NG = 2                    # PSUM evacuation groups per j (4 tiles each)

_BUILT = None
LAST_RESULTS = []   # kept for the test harness's profiling hook


def _k5_body(tc, out_d, st_d, xs_d):
    """Fused single-launch banded-EDT device pass.

    xs_d:  [16, HE, 512] f16 dram (ExternalInput, host-binarized h-slab)
    out_d: [512, 16, HS] f16 dram (ExternalOutput), squared distances,
           w-major.  Rows h in {0,1,62,63} of each slab carry garbage (no
    halo); the host re-solves them from st_d.
    st_d:  [512, 16, 8] f16 dram (ExternalOutput): the squared W-distances
           of rows h in {0..3, 60..63} (what the host boundary fix needs).
    """
    import concourse.mybir as mybir

    nc = tc.nc
    f16 = mybir.dt.float16
    Alu = mybir.AluOpType
    Act = mybir.ActivationFunctionType

    from concourse.masks import make_identity

    with tc.tile_pool(name="const", bufs=1) as cpool, \
         tc.tile_pool(name="big", bufs=1) as bpool, \
         tc.tile_pool(name="htmp", bufs=12) as hpool, \
         tc.tile_pool(name="btmp", bufs=6) as tbpool, \
         tc.tile_pool(name="psum", bufs=4, space="PSUM") as ppool, \
         tc.tile_pool(name="psumw", bufs=1, space="PSUM") as ppoolw:

        ones = cpool.tile([P, W], f16)
        nc.gpsimd.memset(ones[:], 1.0)
        ident = cpool.tile([P, P], f16)
        make_identity(nc, ident[:])
        # dummy transpose so PE observes the gpsimd-built identity before the
        # real transposes (keeps every matmul at <= 1 semaphore wait)
        psw = ppoolw.tile([P, P], f16)
        nc.tensor.transpose(psw[:], ident[:], ident[:])

        AALL = bpool.tile([P, N_T * W], f16)    # d0 = (x != 0) * CLAMP (host)
        FALL = bpool.tile([P, N_T * W], f16)    # fwd scan
        DALL = bpool.tile([P, N_T * W], f16)    # bwd scan of fwd = 1D dist

        # input DMAs on the sync queue (its trigger issue starts immediately;
        # the Pool queue is busy building the identity).  The host sends the
        # already-binarized (x != 0) * CLAMP field, so the first scan starts
        # as soon as the first (single-tile) chunk lands.
        xflat = xs_d.rearrange("b h w -> (b h) w")
        chunks = [(0, 1), (1, 2), (3, 2), (5, 3)]
        for t0, k in chunks:
            if k == 1:
                nc.sync.dma_start(AALL[:, W * t0: W * (t0 + 1)],
                                  xflat[P * t0: P * (t0 + 1)])
            else:
                nc.sync.dma_start(
                    AALL[:, W * t0: W * (t0 + k)].rearrange(
                        "p (g w) -> p g w", g=k),
                    xflat[P * t0: P * (t0 + k)].rearrange(
                        "(g pp) w -> pp g w", g=k))

        for t in range(N_T):
            fa = FALL[:, W * t: W * (t + 1)]
            nc.vector.tensor_tensor_scan(
                fa, ones[:, 0:W], AALL[:, W * t: W * (t + 1)], CLAMP,
                Alu.add, Alu.min)
            nc.vector.tensor_tensor_scan(
                DALL[:, W * t: W * (t + 1)][:, ::-1], ones[:, 0:W],
                fa[:, ::-1], CLAMP, Alu.add, Alu.min)

        # transpose + evacuate-with-Square, in NG groups of 4 scan tiles per
        # j-group; group g holds exactly the b-half g (4*128 = 8*64 lines).
        SQ = bpool.tile([P, N_J * CE], f16)     # w lines x (j, b, h64)
        GT = N_T // NG                          # 4 tiles per group
        for g in range(NG):
            for j in range(N_J):
                ps = ppool.tile([P, GT * P], f16, tag="ps")
                for tt in range(GT):
                    t = g * GT + tt
                    nc.tensor.transpose(
                        ps[:, P * tt: P * (tt + 1)],
                        DALL[:, W * t + P * j: W * t + P * (j + 1)],
                        ident[:])
                nc.scalar.activation(
                    SQ[:, CE * j + GT * P * g: CE * j + GT * P * (g + 1)],
                    ps[:], Act.Square)

        sq5 = SQ[:].rearrange("p (j b h) -> p j b h", j=N_J, b=B)
        ACH = bpool.tile([P, N_J * C], f16)
        ah4 = ACH[:].rearrange("p (j b h) -> p j b h", j=N_J, b=B)
        ACC = bpool.tile([P, N_J * C], f16)
        ac4 = ACC[:].rearrange("p (j b h) -> p j b h", j=N_J, b=B)
        # rows h in {0,1,62,63} are host-re-solved; give them a defined value
        # so the B pass math on those columns stays finite.
        nc.gpsimd.memset(ACH[:], 1024.0)

        # the squared-W-distance strips the host boundary fix needs: rows
        # h in {0..3} and {60..63}.  SWDGE (Pool queue) keeps these off the
        # 8 HWDGE lanes; they are host-bound, not device-critical.
        stv = st_d.rearrange("(j p) b e -> p j b e", p=P)
        for j in range(N_J):
            nc.gpsimd.dma_start(stv[:, j, :, 0:4], sq5[:, j, :, 0:4])
            nc.gpsimd.dma_start(stv[:, j, :, 4:8], sq5[:, j, :, HS - 4:HS])

        # b-halves align with the two PSUM evacuation groups.
        BCH = [(0, 8), (8, 16)]
        HV = HS - 2 * R           # 60 valid output rows, h in [R, HS-R)

        def h_pairs(bc, jh):
            """Pass H pair mins (DVE) + in-place +s^2 (ACT) on one
            (b-half, j-half).  All pairs are emitted before any fold so the
            DVE fold chains never starve."""
            b0, b1 = BCH[bc]
            nb = b1 - b0
            sq = sq5[:, 2 * jh:2 * (jh + 1), b0:b1, :]
            ts = []
            for s in range(1, R + 1):
                lo = sq[:, :, :, R - s:R - s + HV]
                hi = sq[:, :, :, R + s:R + s + HV]
                t_ = hpool.tile([P, 2 * nb * HV], f16, tag="hq")
                tv = t_[:].rearrange("p (j b h) -> p j b h", j=2, b=nb)
                nc.vector.tensor_tensor(tv, lo, hi, Alu.min)
                ts.append(tv)
            for s in range(1, R + 1):
                nc.scalar.activation(ts[s - 1], ts[s - 1], Act.Copy,
                                     bias=float(s * s))
            return ts

        def h_folds(bc, jh, ts):
            """Pass H fold chain (DVE) on one (b-half, j-half)."""
            b0, b1 = BCH[bc]
            sq = sq5[:, 2 * jh:2 * (jh + 1), b0:b1, :]
            a = ah4[:, 2 * jh:2 * (jh + 1), b0:b1, R:R + HV]
            ctr = sq[:, :, :, R:R + HV]
            nc.vector.tensor_tensor(a, ts[0], ctr, Alu.min)
            for s in range(2, R + 1):
                nc.vector.tensor_tensor(a, ts[s - 1], a, Alu.min)

        def b_adds(j0, nj):
            """ACT-side prep for pass B on j-groups [j0, j0+nj): the b = B-1
            strip of the accumulator (its center term) and the shared +s^2
            tensors for s >= 2 (s = 1 is a DVE 4x tensor_scalar in
            b_folds)."""
            a = ah4[:, j0:j0 + nj]
            c = ac4[:, j0:j0 + nj]
            nc.scalar.activation(c[:, :, B - 1:B, :], a[:, :, B - 1:B, :],
                                 Act.Copy, bias=0.0)
            tbs = []
            for s in range(2, R + 1):
                tb = tbpool.tile([P, nj * C], f16, tag=f"tb{nj}")
                tv = tb[:].rearrange("p (j b h) -> p j b h", j=nj, b=B)
                nc.scalar.activation(tv, a, Act.Copy, bias=float(s * s))
                tbs.append(tv)
            return tbs

        def b_folds(j0, nj, tbs):
            """Pass B directional folds on j-groups [j0, j0+nj) (DVE).  The
            s=1 add runs on DVE (4x) so the chain starts without ACT."""
            a = ah4[:, j0:j0 + nj]
            c = ac4[:, j0:j0 + nj]
            tb1 = tbpool.tile([P, nj * C], f16, tag=f"tbd{nj}")
            t1 = tb1[:].rearrange("p (j b h) -> p j b h", j=nj, b=B)
            nc.vector.tensor_scalar(t1, a, 1.0, None, Alu.add)
            for s in range(1, R + 1):
                tv = t1 if s == 1 else tbs[s - 2]
                bc = B - s
                if s == 1:
                    nc.vector.tensor_tensor(c[:, :, 0:bc, :],
                                            tv[:, :, s:B, :],
                                            a[:, :, 0:bc, :], Alu.min)
                else:
                    nc.vector.tensor_tensor(c[:, :, 0:bc, :],
                                            tv[:, :, s:B, :],
                                            c[:, :, 0:bc, :], Alu.min)
                nc.vector.tensor_tensor(c[:, :, s:B, :], tv[:, :, 0:bc, :],
                                        c[:, :, s:B, :], Alu.min)

        outd = out_d.rearrange("(j p) b h -> p j (b h)", p=P)
        accs = ACC[:].rearrange("p (j c) -> p j c", j=N_J)

        # DVE order keeps the engine stall-free: every chunk's pairs first
        # (they only depend on evacuations), then the jh0 fold chains, the
        # jh0 B pass + its DMA, the jh1 fold chains, then per-j B chains
        # each followed by its own DMA so the tail drains incrementally.
        ts = {}
        for bc, jh in [(0, 0), (1, 0), (0, 1), (1, 1)]:
            ts[(bc, jh)] = h_pairs(bc, jh)
        h_folds(0, 0, ts[(0, 0)])
        h_folds(1, 0, ts[(1, 0)])
        tbs0 = b_adds(0, 2)
        b_folds(0, 2, tbs0)
        nc.scalar.dma_start(outd[:, 0:2], accs[:, 0:2])
        h_folds(0, 1, ts[(0, 1)])
        h_folds(1, 1, ts[(1, 1)])
        tbs2 = b_adds(2, 1)
        b_folds(2, 1, tbs2)
        nc.sync.dma_start(outd[:, 2:3], accs[:, 2:3])
        tbs3 = b_adds(3, 1)
        b_folds(3, 1, tbs3)
        nc.sync.dma_start(outd[:, 3:4], accs[:, 3:4])


def _split_multi_waits(nc):
    """Walrus in this toolchain encodes at most ONE sync wait per hardware
    instruction.  Hoist extra waits onto same-engine NoOp carriers inserted
    immediately before the over-subscribed instruction (program order on the
    engine preserves the semantics exactly)."""
    import concourse.mybir as mybir

    n = 0
    for fn in nc.m.functions:
        for blk in fn.blocks:
            insts = blk.instructions
            out = []
            for inst in insts:
                si = inst.sync_info
                if si is not None and len(si.on_wait) > 1:
                    waits = list(si.on_wait)
                    for w in waits[:-1]:
                        nop = mybir.InstNoOp(
                            name=f"waitsplit-{n}", ins=[], outs=[])
                        n += 1
                        nop.engine = inst.engine
                        nop.sync_info = mybir.SyncInfo(
                            on_wait=[w], on_update=[])
                        out.append(nop)
                    inst.sync_info = mybir.SyncInfo(
                        on_wait=[waits[-1]], on_update=list(si.on_update))
                out.append(inst)
            blk.instructions = out
    return n


def _make_tc_class():
    """TileContext whose kernel-tail drain is split into one drain per proc.

    The stock tail emits a single sync-engine Drain waiting on every
    outstanding processor; this walrus build only encodes ONE sync wait per
    instruction, so the aggregated drain fails codegen.  Semantics are
    identical — the waits just land on consecutive Drain instructions.
    """
    import concourse.tile as tile
    from concourse.vector_clock import ScopedClock, VectorClock

    class SplitDrainTileContext(tile.TileContext):
        def _drain_and_barrier(self, tick_clock, wait_clock):
            gvc = tick_clock.global_clock
            for proc in range(len(gvc)):
                t = gvc[proc]
                if t <= 0:
                    continue
                d = self.nc.sync.drain()
                sv = VectorClock([0] * len(gvc))
                sv.require_at_least(proc, t)
                wait_clock.add_sem_waits(d.ins, ScopedClock({None: sv}))
            self.nc.all_engine_barrier()
            assert self.sems is not None
            popped = self.nc._tile_sem_poison_stack.pop()
            assert popped is self._sem_poison
            self.nc.clear_and_free_semaphores(
                list(self.sems.allocated().values()))
            self.nc.all_engine_barrier()

    return SplitDrainTileContext


def _build():
    """Build the fused Bass module (done once per process)."""
    import concourse.bass as bass
    import concourse.mybir as mybir

    f16 = mybir.dt.float16
    TC = _make_tc_class()

    nc5 = bass.Bass("TRN2", debug=False, num_devices=NCORES)
    xs5_d = nc5.dram_tensor("xs5", [B, HE, W], f16,
                            kind="ExternalInput").ap()
    ot5_d = nc5.dram_tensor("ot5", [W, B, HS], f16,
                            kind="ExternalOutput").ap()
    st5_d = nc5.dram_tensor("st5", [W, B, 8], f16,
                            kind="ExternalOutput").ap()
    with TC(nc5) as tc:
        _k5_body(tc, ot5_d, st5_d, xs5_d)
    _split_multi_waits(nc5)
    return (nc5,)


def _host_exact_edt(x):
    """Exact host fallback: banded numpy EDT with growing radius (f32)."""
    INF = np.float32(1e9)
    r = 8
    while True:
        d0 = np.where(x != 0, INF, np.float32(0.0))
        fwd = np.empty_like(d0)
        st = np.full(d0.shape[:2], INF, np.float32)
        for w in range(W):
            st = np.minimum(st + 1.0, d0[:, :, w]); fwd[:, :, w] = st
        st = np.full(d0.shape[:2], INF, np.float32)
        bwd = np.empty_like(d0)
        for w in range(W - 1, -1, -1):
            st = np.minimum(st + 1.0, d0[:, :, w]); bwd[:, :, w] = st
        d2 = np.minimum(fwd, bwd) ** 2
        for axis in (0, 1):
            src = d2
            acc = src.copy()
            rr = min(r, x.shape[axis] - 1)
            for s in range(1, rr + 1):
                sl_lo = [slice(None)] * 3
                sl_hi = [slice(None)] * 3
                sl_lo[axis] = slice(0, x.shape[axis] - s)
                sl_hi[axis] = slice(s, None)
                np.minimum(acc[tuple(sl_lo)], src[tuple(sl_hi)] + s * s,
                           out=acc[tuple(sl_lo)])
                np.minimum(acc[tuple(sl_hi)], src[tuple(sl_lo)] + s * s,
                           out=acc[tuple(sl_hi)])
            d2 = acc
        out = np.sqrt(d2)
        # exact when every per-axis offset fits in the band; r >= max dim
        # means the bands are complete regardless of the value of out
        if out.max() <= r or r >= max(x.shape):
            return out.astype(np.float32)
        r *= 2


_RUNNER = None


def _make_runner(nc, n_cores):
    """Build the sharded PJRT callable once (run_bass_kernel_spmd re-traces
    and re-jits on every call; caching saves ~1 s per kernel() invocation)."""
    import jax
    import numpy as _np
    from jax.sharding import Mesh, PartitionSpec
    from jax.experimental.shard_map import shard_map
    import concourse.mybir as mybir
    from concourse import bass2jax

    bass2jax.install_neuronx_cc_hook()
    partition_name = (nc.partition_id_tensor.name
                      if nc.partition_id_tensor else None)
    in_names, out_names, out_avals, zero_outs = [], [], [], []
    for alloc in nc.m.functions[0].allocations:
        if not isinstance(alloc, mybir.MemoryLocationSet):
            continue
        name = alloc.memorylocations[0].name
        if alloc.kind == "ExternalInput":
            if name != partition_name:
                in_names.append(name)
        elif alloc.kind == "ExternalOutput":
            out_avals.append(jax.core.ShapedArray(
                tuple(alloc.tensor_shape), mybir.dt.np(alloc.dtype)))
            out_names.append(name)
            zero_outs.append(_np.zeros(tuple(alloc.tensor_shape),
                                       mybir.dt.np(alloc.dtype)))
    all_in = list(in_names) + list(out_names)
    if partition_name is not None:
        all_in.append(partition_name)

    def _body(*args):
        operands = list(args)
        if partition_name is not None:
            operands.append(bass2jax.partition_id_tensor())
        return tuple(bass2jax._bass_exec_p.bind(
            *operands, out_avals=tuple(out_avals), in_names=tuple(all_in),
            out_names=tuple(out_names), lowering_input_output_aliases=(),
            sim_require_finite=True, sim_require_nnan=True, nc=nc))

    devices = jax.devices()[:n_cores]
    mesh = Mesh(_np.asarray(devices), ("core",))
    n_io = len(in_names) + len(out_names)
    fn = jax.jit(shard_map(_body, mesh=mesh,
                           in_specs=(PartitionSpec("core"),) * n_io,
                           out_specs=(PartitionSpec("core"),) * len(out_names),
                           check_rep=False), keep_unused=True)

    def run(in_maps):
        concat_in = [_np.concatenate([_np.asarray(in_maps[c][n])
                                      for c in range(n_cores)], axis=0)
                     for n in in_names]
        concat_zero = [_np.zeros((n_cores * z.shape[0], *z.shape[1:]), z.dtype)
                       for z in zero_outs]
        outs = fn(*concat_in, *concat_zero)
        return [{name: _np.asarray(outs[i]).reshape(
                    n_cores, *out_avals[i].shape)[c]
                 for i, name in enumerate(out_names)}
                for c in range(n_cores)]

    return run


def _fix_boundaries(d2, results):
    """Re-solve the 4 rows around every slab boundary (and the 2 rows at
    each volume edge) from the exported squared-W-distance strips: the
    device ran the H pass without halo, so those rows are garbage.  This is
    the same radius-R H+B min-plus the device does, just in numpy on 32 of
    512 rows."""
    # dw2[b, h, w] for the strip rows: slab k locals {0..3} u {60..63}
    dw2 = np.full((B, H, W), np.float32(np.inf))
    for k in range(NCORES):
        st = np.asarray(results[k]["st5"]).astype(np.float32)  # [W, B, 8]
        st = st.transpose(1, 2, 0)                             # [B, 8, W]
        dw2[:, k * HS:k * HS + 4, :] = st[:, 0:4, :]
        dw2[:, k * HS + HS - 4:k * HS + HS, :] = st[:, 4:8, :]
    rows = sorted({r for m in range(NCORES + 1)
                   for r in (m * HS - 2, m * HS - 1, m * HS, m * HS + 1)
                   if 0 <= r < H})
    for hg in rows:
        acc = None
        for dh in range(-R, R + 1):
            hh = hg + dh
            if not 0 <= hh < H:
                continue
            cand = dw2[:, hh, :] + np.float32(dh * dh)
            acc = cand if acc is None else np.minimum(acc, cand)
        # pass B (radius R) along the batch axis
        accb = acc.copy()
        for db in range(1, R + 1):
            np.minimum(accb[:-db], acc[db:] + db * db, out=accb[:-db])
            np.minimum(accb[db:], acc[:-db] + db * db, out=accb[db:])
        d2[:, hg, :] = accb


def _patch_far(d2, xin):
    """Re-solve every voxel with banded d^2 >= (R+1)^2 exactly via a
    radius-6 window search (any voxel the band could have gotten wrong is in
    this set: the banded value only over-estimates, and a band violation
    implies true distance >= R+1).  Returns (patched d2, ok); ok=False means
    some such voxel has no zero within distance < 6 (or there are
    implausibly many) and the caller must use the full exact fallback."""
    sus = np.argwhere(d2 >= (R + 1) ** 2 - 0.5)
    if sus.shape[0] == 0:
        return d2, True
    if sus.shape[0] > 1_000_000:
        return d2, False
    rr = 6
    zp = np.pad(xin == 0, rr, constant_values=False)
    og = np.arange(-rr, rr + 1, dtype=np.int32)
    ob, oh, ow = np.meshgrid(og, og, og, indexing="ij")
    w2 = (ob * ob + oh * oh + ow * ow).astype(np.float32).ravel()
    obf = (ob.ravel() + rr)[None, :]
    ohf = (oh.ravel() + rr)[None, :]
    owf = (ow.ravel() + rr)[None, :]
    vals = np.empty(sus.shape[0], np.float32)
    CH = 2048
    for i0 in range(0, sus.shape[0], CH):
        s = sus[i0:i0 + CH].astype(np.int32)
        win = zp[s[:, 0:1] + obf, s[:, 1:2] + ohf, s[:, 2:3] + owf]
        d2w = np.where(win, w2[None, :], np.float32(1e9)).min(axis=1)
        if (d2w > 35.5).any():
            return d2, False
        vals[i0:i0 + CH] = d2w
    d2[sus[:, 0], sus[:, 1], sus[:, 2]] = vals
    return d2, True


def kernel(x):
    global _BUILT, _RUNNER
    x = np.asarray(x)
    assert x.shape == (B, H, W)
    if x.dtype != np.float32:
        x = x.astype(np.float32)

    if _BUILT is None:
        _BUILT = _build()
    (nc5,) = _BUILT
    if _RUNNER is None:
        _RUNNER = _make_runner(nc5, NCORES)
    LAST_RESULTS.clear()

    nan_mask = np.isnan(x)
    # The host binarizes: the device receives (x != 0) * CLAMP directly
    # (NaN != 0 is True, so NaN voxels are foreground, as in the reference).
    xin = (x != 0).astype(np.float32)     # 0 at zeros, 1 at foreground/NaN
    xp = (xin * CLAMP).astype(np.float16)
    in5 = [{"xs5": np.ascontiguousarray(xp[:, k * HS:k * HS + HE, :])}
           for k in range(NCORES)]
    results = _RUNNER(in5)
    outt = np.concatenate([results[k]["ot5"] for k in range(NCORES)], axis=2)

    d2 = outt.transpose(1, 2, 0).astype(np.float32)   # (w,b,h) -> (b,h,w)
    _fix_boundaries(d2, results)
    d2, ok = _patch_far(d2, xin)
    out = np.sqrt(d2) if ok else _host_exact_edt(xin)

    if nan_mask.any():
        out = np.where(nan_mask, np.float32(np.nan), out)
    return out
